# revision 1
# baseline (speedup 1.0000x reference)
"""Trainium2 Bass kernel for nn_DiffusionActionHead (B=8, S=2048, D=4096).

Strategy (8 NeuronCores):
  - Data-parallel over batch for everything touching llm_output (32 MiB/core).
  - Tensor-parallel weight reads: core i reads column-slice i of wq/wk/wv,
    row-slice i of wo, column/row slice i of mlp_w1/mlp_w2 (~96 MiB of
    weights split 8 ways), tiny diffusion tail replicated.
  - MAP-head attention with q_len=1 is collapsed algebraically:
        scores[s,h] = llm[s,:] . U[:,h],   U[:,h] = wk[:,hb] @ q_h / sqrt(DH)
        pooled[h,:] = softmax(scores)[h,:] @ llm
        ctx[hb]     = pooled[h,:] @ wv[:,hb] + bv[hb]
    (bk shifts scores by a per-head constant -> cancels in softmax.)
  - 4 small collectives: AllGather(U cols), AllToAll(pooled, head<->batch),
    AllReduce(attn_out partial), AllReduce(mlp partial).
  - Large matmuls run in fp16 (1 cyc/row on PE, half the HBM bytes); all
    accumulation, softmax, layernorms and residuals stay fp32.
  - Biases are folded into the PSUM accumulations via a ones-row matmul;
    additive biases of AllReduce'd partials are pre-divided by 8 on host.
  - Activations are kept feature-on-partition ("transposed") so every big
    matmul streams its weight slice in natural DRAM layout as the moving
    operand; llm itself is passed in both layouts (llmT host-transposed).
  - Two HWDGE queues: sync carries the llm streams, scalar carries the
    weight streams, so a stalled stream never head-of-line-blocks the other.
"""

import numpy as np
import sys

if "/opt/trn_rl_repo" not in sys.path:
    sys.path.insert(0, "/opt/trn_rl_repo")

import concourse.bass as bass
import concourse.tile as tile
from concourse import bacc, mybir
from concourse.masks import make_identity
from concourse.bass_utils import run_bass_kernel_spmd

F32 = mybir.dt.float32
F16 = mybir.dt.float16
AF = mybir.ActivationFunctionType
ALU = mybir.AluOpType

B, S, D = 8, 2048, 4096
H, AD, TD, HID, NBLK = 8, 7, 32, 256, 3
DH = D // H            # 512
NC = 8                 # cores
P = 128
SC = S // P            # 16 S-chunks
DC = D // P            # 32 D-chunks
HD2 = D // 2           # 2048 (half width -> 4-bank PSUM tiles)
F1S = 4 * D // NC      # 2048 per-core hidden cols of mlp_w1
HC = HID // P          # 2
RSQRT_DH = 1.0 / float(np.sqrt(DH))
TWO_PI = 2.0 * float(np.pi)


def _bcast(src_ap, nparts):
    """Partition-broadcast a (1, N) DRAM AP to (nparts, N)."""
    ap = src_ap
    assert ap.shape[0] == 1, ap.shape
    return bass.AP(tensor=ap.tensor, offset=ap.offset,
                   ap=[[0, nparts]] + [list(x) for x in ap.ap[1:]])


def build_program():
    nc = bacc.Bacc("TRN2", target_bir_lowering=False, debug=False,
                   num_devices=NC)
    t = {}

    def din(name, shape, dtype=F32):
        t[name] = nc.dram_tensor(name, shape, dtype, kind="ExternalInput")

    din("llm", [S, D], F16); din("llmT", [D, S], F16)
    din("wq_s", [D, DH], F16); din("bq_s", [1, DH])
    din("wkT_s", [DH, D], F16)
    din("wv_s", [D, DH], F16); din("bv16", [1, DH], F16)
    din("wo_s", [DH, D], F16); din("bo16", [1, D], F16)        # bo/8
    din("ln_g_r", [P, DC]); din("ln_b_r", [P, DC])
    din("w1_s", [D, F1S], F16); din("b116", [1, F1S], F16)
    din("w2_s", [F1S, D], F16); din("b216", [1, D], F16)       # b2/8
    din("probe_r", [P, DC], F16)
    din("four_w2", [TD, 1]); din("phase2", [TD, 1])
    din("timeT", [1, B]); din("naT", [AD, B], F16)
    din("cond_w1", [TD, 2 * TD], F16); din("cond_b1c", [2 * TD, 1])
    din("cond_w2", [2 * TD, TD], F16); din("cond_b2c", [TD, 1])
    din("rin_cond", [TD, HID], F16); din("rin_pool", [D, HID], F16)
    din("rin_na", [AD, HID], F16); din("rb16", [1, HID], F16)
    din("blk_g_r", [NBLK, P, HC]); din("blk_b_r", [NBLK, P, HC])
    din("blk_w1", [NBLK, HID, 4 * HID], F16)
    din("blk_b1_16", [NBLK, 4 * HID], F16)
    din("blk_w2", [NBLK, 4 * HID, HID], F16)
    din("blk_b2_16", [NBLK, HID], F16)
    din("out_w", [HID, AD], F16); din("out_bc", [1, AD])
    t["res"] = nc.dram_tensor("res", [B, AD], F32, kind="ExternalOutput")

    # collective bounce buffers (internal DRAM; AG/AR outputs in Shared space)
    t["cc_u_in"] = nc.dram_tensor("cc_u_in", [1, D], F32)
    t["cc_u_out"] = nc.dram_tensor("cc_u_out", [NC, D], F32, addr_space="Shared")
    t["cc_pool_in"] = nc.dram_tensor("cc_pool_in", [H, D], F32)
    t["cc_pool_out"] = nc.dram_tensor("cc_pool_out", [B, D], F32)
    t["cc_attn_in"] = nc.dram_tensor("cc_attn_in", [B, D], F32)
    t["cc_attn_out"] = nc.dram_tensor("cc_attn_out", [B, D], F32,
                                      addr_space="Shared")
    t["cc_mlp_in"] = nc.dram_tensor("cc_mlp_in", [B, D], F32)
    t["cc_mlp_out"] = nc.dram_tensor("cc_mlp_out", [B, D], F32,
                                     addr_space="Shared")

    with tile.TileContext(nc) as tc:
        import contextlib
        with contextlib.ExitStack() as ctx:
            _build(nc, tc, t, ctx)
    nc.finalize()
    return nc


def _build(nc, tc, t, ctx):
    GROUPS = [list(range(NC))]

    singles = ctx.enter_context(tc.tile_pool(name="singles", bufs=1))
    llm_pool = ctx.enter_context(tc.tile_pool(name="llm_pool", bufs=6))
    llmT_pool = ctx.enter_context(tc.tile_pool(name="llmT_pool", bufs=8))
    wst = ctx.enter_context(tc.tile_pool(name="wst", bufs=8))
    nat16 = ctx.enter_context(tc.tile_pool(name="nat16", bufs=2))
    nat8 = ctx.enter_context(tc.tile_pool(name="nat8", bufs=2))
    psA = ctx.enter_context(tc.tile_pool(name="psA", bufs=1, space="PSUM"))
    psB = ctx.enter_context(tc.tile_pool(name="psB", bufs=2, space="PSUM"))
    psC = ctx.enter_context(tc.tile_pool(name="psC", bufs=2, space="PSUM"))

    ident = singles.tile([P, P], F32)
    make_identity(nc, ident)
    eps_sb = singles.tile([P, 1], F32)
    nc.vector.memset(eps_sb[:], 1e-5)
    ones8 = singles.tile([1, 8], F16)
    nc.vector.memset(ones8[:], 1.0)

    def evict(dst, src):
        nc.vector.tensor_copy(out=dst, in_=src)

    def t_nat_to_T(src_nat, dst_T, nchunks, npart, uid):
        """(npart, nchunks*128) sbuf -> (128, nchunks, npart) sbuf via PE."""
        for c in range(nchunks):
            ps = psB.tile([P, 8], F32, tag="tp8", name=f"tp_{uid}_{c}")
            nc.tensor.transpose(ps[:, :npart], src_nat[:, c * P:(c + 1) * P],
                                ident[:npart, :npart])
            evict(dst_T[:, c, :], ps[:, :npart])

    def bias_mm(ps, bias_row, n_total, stop=True):
        """Add a (1, n_total) f16 bias row into psum (8, n_total) via ones-row
        matmuls, 512 cols per matmul (moving-dim limit)."""
        nch = (n_total + 511) // 512
        for n in range(nch):
            w = min(512, n_total - n * 512)
            nc.tensor.matmul(ps[:, n * 512:n * 512 + w], ones8[:, :B],
                             bias_row[:, n * 512:n * 512 + w],
                             start=False, stop=(stop and n == nch - 1))

    def layernorm_nat(x_nat, npart, n, y_nat, uid):
        """y = (x - mean) / sqrt(var + eps) over the free dim of (npart, n)."""
        nsub = max(1, n // 512)
        st = nat8.tile([npart, nsub, nc.vector.BN_STATS_DIM], F32, tag="lnst",
                       name=f"lnst_{uid}")
        xg = x_nat.rearrange("p (a b) -> p a b", a=nsub)
        for g in range(nsub):
            nc.vector.bn_stats(out=st[:, g, :], in_=xg[:, g, :])
        mv = nat8.tile([npart, nc.vector.BN_AGGR_DIM], F32, tag="lnmv",
                       name=f"lnmv_{uid}")
        nc.vector.bn_aggr(out=mv[:], in_=st[:])
        std = nat8.tile([npart, 1], F32, tag="lnsd", name=f"lnsd_{uid}")
        nc.scalar.activation(out=std[:], in_=mv[:, 1:2], func=AF.Sqrt,
                             bias=eps_sb[:npart, :])
        nc.vector.reciprocal(out=std[:], in_=std[:])
        nc.vector.tensor_scalar(out=y_nat, in0=x_nat, scalar1=mv[:, 0:1],
                                scalar2=std[:], op0=ALU.subtract, op1=ALU.mult)

    # =======================================================================
    # STEP 0: small constants, bias rows, tail weights — all prefetched
    # early on idle queues so the tail phase never waits on them.
    # =======================================================================
    probe_sb = singles.tile([P, DC], F16)
    nc.sync.dma_start(out=probe_sb[:], in_=t["probe_r"][:])
    bq_sb = singles.tile([1, DH], F32)
    nc.sync.dma_start(out=bq_sb[:], in_=t["bq_s"][:])
    bv_sb = singles.tile([1, DH], F16)
    nc.gpsimd.dma_start(out=bv_sb[:], in_=t["bv16"][:])
    bo_sb = singles.tile([1, D], F16)
    nc.gpsimd.dma_start(out=bo_sb[:], in_=t["bo16"][:])
    b1_sb = singles.tile([1, F1S], F16)
    nc.gpsimd.dma_start(out=b1_sb[:], in_=t["b116"][:])
    b2_sb = singles.tile([1, D], F16)
    nc.gpsimd.dma_start(out=b2_sb[:], in_=t["b216"][:])
    rb_sb = singles.tile([1, HID], F16)
    nc.gpsimd.dma_start(out=rb_sb[:], in_=t["rb16"][:])
    bb1_sb = singles.tile([1, NBLK, 4 * HID], F16)
    nc.gpsimd.dma_start(out=bb1_sb[:], in_=t["blk_b1_16"][:].rearrange("n f -> (n f)")[None, :])
    bb2_sb = singles.tile([1, NBLK, HID], F16)
    nc.gpsimd.dma_start(out=bb2_sb[:], in_=t["blk_b2_16"][:].rearrange("n f -> (n f)")[None, :])
    lng_sb = singles.tile([P, DC], F32)
    nc.sync.dma_start(out=lng_sb[:], in_=t["ln_g_r"][:])
    lnb_sb = singles.tile([P, DC], F32)
    nc.sync.dma_start(out=lnb_sb[:], in_=t["ln_b_r"][:])
    bgr_sb = singles.tile([P, NBLK, HC], F32)
    nc.sync.dma_start(out=bgr_sb[:],
                      in_=t["blk_g_r"][:].rearrange("n p c -> p n c"))
    bbr_sb = singles.tile([P, NBLK, HC], F32)
    nc.sync.dma_start(out=bbr_sb[:],
                      in_=t["blk_b_r"][:].rearrange("n p c -> p n c"))
    rc_sb = singles.tile([TD, HID], F16)
    nc.scalar.dma_start(out=rc_sb[:], in_=t["rin_cond"][:])
    rna_sb = singles.tile([AD, HID], F16)
    nc.scalar.dma_start(out=rna_sb[:], in_=t["rin_na"][:])
    naT_sb = singles.tile([AD, B], F16)
    nc.sync.dma_start(out=naT_sb[:], in_=t["naT"][:])
    ow_sb = singles.tile([P, HC, AD], F16)
    nc.sync.dma_start(out=ow_sb[:],
                      in_=t["out_w"][:].rearrange("(c p) a -> p c a", p=P))
    ob_bc = singles.tile([B, AD], F32)
    nc.gpsimd.dma_start(out=ob_bc[:], in_=_bcast(t["out_bc"][:], B))

    # =======================================================================
    # STEP 1: q = (probe @ wq_s + bq) / sqrt(DH)    -> (1, 512) natural
    # wq is streamed in 8 half-MiB DMAs (4 k-chunks each) on the scalar ring.
    # =======================================================================
    q_nat = singles.tile([1, DH], F32)
    ps_q = psC.tile([1, DH], F32, tag="vec", name="ps_q")
    wq_r = t["wq_s"].rearrange("(c p) n -> p c n", p=P)
    for g in range(8):
        wt = wst.tile([P, 4, DH], F16, tag="wst", name=f"wq_g{g}")
        nc.scalar.dma_start(out=wt[:], in_=wq_r[:, 4 * g:4 * g + 4, :])
        for j in range(4):
            k = 4 * g + j
            nc.tensor.matmul(ps_q[:], probe_sb[:, k:k + 1], wt[:, j, :],
                             start=(k == 0), stop=(k == DC - 1))
    nc.vector.tensor_add(out=q_nat[:], in0=ps_q[:], in1=bq_sb[:])
    nc.vector.tensor_scalar_mul(out=q_nat[:], in0=q_nat[:], scalar1=RSQRT_DH)

    qT = singles.tile([P, DH // P], F16)  # (128, 4)
    for c in range(DH // P):
        ps = psB.tile([P, 8], F32, tag="tp8", name=f"tp_q_{c}")
        nc.tensor.transpose(ps[:, :1], q_nat[:, c * P:(c + 1) * P], ident[:1, :1])
        evict(qT[:, c:c + 1], ps[:, :1])

    # =======================================================================
    # STEP 2: U column of this core's head: U = wkT_s.T @ q~  -> (1, 4096)
    #         AllGather -> cc_u_out (8, 4096) = U.T with one row per head
    # =======================================================================
    u_nat = nat16.tile([1, D], F32, tag="nat16", name="u_nat")
    for nhalf in range(2):
        wk_tiles = []
        for k in range(DH // P):
            wt = wst.tile([P, HD2], F16, tag="wst", name=f"wk_t{nhalf}_{k}")
            nc.scalar.dma_start(
                out=wt[:],
                in_=t["wkT_s"][k * P:(k + 1) * P, nhalf * HD2:(nhalf + 1) * HD2])
            wk_tiles.append(wt)
        for ncol in range(4):
            n0 = nhalf * 4 + ncol
            ps_u = psC.tile([1, DH], F32, tag="vec", name=f"ps_u_{n0}")
            for k in range(DH // P):
                nc.tensor.matmul(
                    ps_u[:], qT[:, k:k + 1],
                    wk_tiles[k][:, ncol * DH:(ncol + 1) * DH],
                    start=(k == 0), stop=(k == DH // P - 1))
            evict(u_nat[:, n0 * DH:(n0 + 1) * DH], ps_u[:])

    nc.gpsimd.dma_start(out=t["cc_u_in"][:], in_=u_nat[:])
    nc.gpsimd.collective_compute(
        "AllGather", ALU.bypass, replica_groups=GROUPS,
        ins=[t["cc_u_in"][:].opt()], outs=[t["cc_u_out"][:].opt()])

    # ---- cond path (fourier + tiny mlp) — independent of everything above,
    # computed here so it is off the critical path of the tail.
    fw_sb = singles.tile([TD, 1], F32)
    nc.sync.dma_start(out=fw_sb[:], in_=t["four_w2"][:])
    ph_sb = singles.tile([TD, 1], F32)
    nc.sync.dma_start(out=ph_sb[:], in_=t["phase2"][:])
    tb32 = singles.tile([TD, B], F32)
    nc.gpsimd.dma_start(out=tb32[:], in_=_bcast(t["timeT"][:], TD))
    fu = singles.tile([TD, B], F32)
    nc.vector.tensor_scalar_mul(out=fu[:], in0=tb32[:], scalar1=fw_sb[:])
    # exact range reduction: sin/cos have period 1 in fu, so subtract the
    # integer part via an f32->i32->f32 round-trip (|fu| < ~64 here).
    fi = singles.tile([TD, B], mybir.dt.int32)
    nc.vector.tensor_copy(out=fi[:], in_=fu[:])
    fif = singles.tile([TD, B], F32)
    nc.vector.tensor_copy(out=fif[:], in_=fi[:])
    nc.vector.tensor_sub(out=fu[:], in0=fu[:], in1=fif[:])
    ffT = singles.tile([TD, B], F16)
    nc.scalar.activation(out=ffT[:], in_=fu[:], func=AF.Sin,
                         scale=TWO_PI, bias=ph_sb[:])
    cw1_sb = singles.tile([TD, 2 * TD], F16)
    nc.scalar.dma_start(out=cw1_sb[:], in_=t["cond_w1"][:])
    cb1_sb = singles.tile([2 * TD, 1], F32)
    nc.sync.dma_start(out=cb1_sb[:], in_=t["cond_b1c"][:])
    cw2_sb = singles.tile([2 * TD, TD], F16)
    nc.scalar.dma_start(out=cw2_sb[:], in_=t["cond_w2"][:])
    cb2_sb = singles.tile([TD, 1], F32)
    nc.sync.dma_start(out=cb2_sb[:], in_=t["cond_b2c"][:])
    ps_c1 = psB.tile([P, 8], F32, tag="tp8", name="ps_c1")
    nc.tensor.matmul(ps_c1[:2 * TD, :B], cw1_sb[:], ffT[:], start=True, stop=True)
    c1 = singles.tile([2 * TD, B], F16)
    nc.scalar.activation(out=c1[:], in_=ps_c1[:2 * TD, :B], func=AF.Silu,
                         bias=cb1_sb[:])
    ps_c2 = psB.tile([P, 8], F32, tag="tp8", name="ps_c2")
    nc.tensor.matmul(ps_c2[:TD, :B], cw2_sb[:], c1[:], start=True, stop=True)
    condT = singles.tile([TD, B], F16)
    nc.scalar.activation(out=condT[:], in_=ps_c2[:TD, :B], func=AF.Identity,
                         bias=cb2_sb[:])

    # ---- read back U.T (8, 4096), transpose to (128, 32, 8), cast to f16
    uh_nat = nat16.tile([H, D], F32, tag="nat16", name="uh_nat")
    nc.sync.dma_start(out=uh_nat[:], in_=t["cc_u_out"][:])
    u_f16 = singles.tile([P, DC, H], F16)
    for c in range(DC):
        ps = psB.tile([P, 8], F32, tag="tp8", name=f"tp_u_{c}")
        nc.tensor.transpose(ps[:, :H], uh_nat[:, c * P:(c + 1) * P],
                            ident[:H, :H])
        evict(u_f16[:, c, :], ps[:, :H])

    # =======================================================================
    # STEP 3: scoresT (8, 2048) = U.T @ llmT  (fp16 inputs, fp32 accum)
    # =======================================================================
    ps_sc = psA.tile([H, S], F32, tag="big", name="ps_sc")
    for k in range(DC):
        lt = llmT_pool.tile([P, S], F16, tag="llmT", name=f"llmT_t{k}")
        nc.sync.dma_start(out=lt[:], in_=t["llmT"][k * P:(k + 1) * P, :])
        for n in range(S // 512):
            nc.tensor.matmul(ps_sc[:, n * 512:(n + 1) * 512],
                             u_f16[:, k, :], lt[:, n * 512:(n + 1) * 512],
                             start=(k == 0), stop=(k == DC - 1))

    # =======================================================================
    # STEP 4: softmax over S. Max-subtraction is skipped deliberately:
    # softmax is shift-invariant and |scores| here is < ~1, so exp() is
    # perfectly conditioned; the result is mathematically identical.
    # =======================================================================
    p_nat = nat8.tile([H, S], F32, tag="nat8", name="p_nat")
    nc.scalar.activation(out=p_nat[:], in_=ps_sc[:], func=AF.Exp)
    den = singles.tile([H, 1], F32)
    nc.vector.reduce_sum(out=den[:], in_=p_nat[:], axis=mybir.AxisListType.X)
    nc.vector.reciprocal(out=den[:], in_=den[:])
    nc.vector.tensor_scalar_mul(out=p_nat[:], in0=p_nat[:], scalar1=den[:])
    pT = singles.tile([P, SC, H], F16)
    t_nat_to_T(p_nat, pT, SC, H, "p")

    # =======================================================================
    # STEP 5: pooled (8, 4096) = pT.T @ llm ; AllToAll (head <-> batch)
    # =======================================================================
    pooled_nat = nat16.tile([H, D], F32, tag="nat16", name="pooled_nat")
    for half in range(2):
        ps_p = psA.tile([H, HD2], F32, tag="big", name=f"ps_pool_{half}")
        for s in range(SC):
            lt = llm_pool.tile([P, HD2], F16, tag="llm", name=f"llm_t{half}_{s}")
            nc.sync.dma_start(
                out=lt[:],
                in_=t["llm"][s * P:(s + 1) * P, half * HD2:(half + 1) * HD2])
            for n in range(HD2 // 512):
                nc.tensor.matmul(ps_p[:, n * 512:(n + 1) * 512],
                                 pT[:, s, :], lt[:, n * 512:(n + 1) * 512],
                                 start=(s == 0), stop=(s == SC - 1))
        evict(pooled_nat[:, half * HD2:(half + 1) * HD2], ps_p[:])

    nc.gpsimd.dma_start(out=t["cc_pool_in"][:], in_=pooled_nat[:])
    nc.gpsimd.collective_compute(
        "AllToAll", ALU.bypass, replica_groups=GROUPS,
        ins=[t["cc_pool_in"][:].opt()], outs=[t["cc_pool_out"][:].opt()])

    # =======================================================================
    # STEP 6: ctx for this core's head, all batches: (8, 512) = poolh@wv + bv
    # =======================================================================
    poolh_nat = nat16.tile([B, D], F32, tag="nat16", name="poolh_nat")
    nc.sync.dma_start(out=poolh_nat[:], in_=t["cc_pool_out"][:])
    poolhT = singles.tile([P, DC, B], F16)
    t_nat_to_T(poolh_nat, poolhT, DC, B, "ph")

    ps_cx = psA.tile([B, DH], F32, tag="big", name="ps_cx")
    wv_r = t["wv_s"].rearrange("(c p) n -> p c n", p=P)
    for g in range(8):
        wt = wst.tile([P, 4, DH], F16, tag="wst", name=f"wv_g{g}")
        nc.scalar.dma_start(out=wt[:], in_=wv_r[:, 4 * g:4 * g + 4, :])
        for j in range(4):
            k = 4 * g + j
            nc.tensor.matmul(ps_cx[:], poolhT[:, k, :], wt[:, j, :],
                             start=(k == 0), stop=False)
    bias_mm(ps_cx, bv_sb, DH)
    ctx_nat = nat8.tile([B, DH], F32, tag="nat8", name="ctx_nat")
    evict(ctx_nat[:], ps_cx[:])
    ctxT = singles.tile([P, DH // P, B], F16)
    t_nat_to_T(ctx_nat, ctxT, DH // P, B, "cx")

    # =======================================================================
    # STEP 7: attn partial (8, 4096) = ctx @ wo_s + bo/8 ; AllReduce
    # =======================================================================
    attn_part = nat16.tile([B, D], F32, tag="nat16", name="attn_part")
    for half in range(2):
        ps_a = psA.tile([B, HD2], F32, tag="big", name=f"ps_attn_{half}")
        for k in range(DH // P):
            wt = wst.tile([P, HD2], F16, tag="wst", name=f"wo_t{half}_{k}")
            nc.scalar.dma_start(
                out=wt[:],
                in_=t["wo_s"][k * P:(k + 1) * P, half * HD2:(half + 1) * HD2])
            for n in range(HD2 // 512):
                nc.tensor.matmul(ps_a[:, n * 512:(n + 1) * 512],
                                 ctxT[:, k, :], wt[:, n * 512:(n + 1) * 512],
                                 start=(k == 0), stop=False)
        bias_mm(ps_a, bo_sb[:, half * HD2:(half + 1) * HD2], HD2)
        evict(attn_part[:, half * HD2:(half + 1) * HD2], ps_a[:])
    nc.gpsimd.dma_start(out=t["cc_attn_in"][:], in_=attn_part[:])
    nc.gpsimd.collective_compute(
        "AllReduce", ALU.add, replica_groups=GROUPS,
        ins=[t["cc_attn_in"][:].opt()], outs=[t["cc_attn_out"][:].opt()])

    # =======================================================================
    # STEP 8: y = LN(attn_out)*g+b ; mlp partial (+b1, gelu, @w2 + b2/8) ; AR
    # =======================================================================
    attn_nat = singles.tile([B, D], F32)  # persists (residual)
    nc.sync.dma_start(out=attn_nat[:], in_=t["cc_attn_out"][:])

    y_nat = nat16.tile([B, D], F32, tag="nat16", name="y_nat")
    layernorm_nat(attn_nat[:], B, D, y_nat[:], "ln0")
    yT = singles.tile([P, DC, B], F16)
    t_nat_to_T(y_nat, yT, DC, B, "y")
    # LN affine in T layout (gamma/beta become per-partition scalars)
    for c in range(DC):
        nc.vector.tensor_scalar(out=yT[:, c, :], in0=yT[:, c, :],
                                scalar1=lng_sb[:, c:c + 1],
                                scalar2=lnb_sb[:, c:c + 1],
                                op0=ALU.mult, op1=ALU.add)

    # mm1: h1 (8, 2048) = y @ w1_s + b1 ; exact gelu straight off PSUM
    ps_h1 = psA.tile([B, F1S], F32, tag="big", name="ps_h1")
    for k in range(DC):
        wt = wst.tile([P, F1S], F16, tag="wst", name=f"w1_t{k}")
        nc.scalar.dma_start(out=wt[:], in_=t["w1_s"][k * P:(k + 1) * P, :])
        for n in range(F1S // 512):
            nc.tensor.matmul(ps_h1[:, n * 512:(n + 1) * 512],
                             yT[:, k, :], wt[:, n * 512:(n + 1) * 512],
                             start=(k == 0), stop=False)
    bias_mm(ps_h1, b1_sb, F1S)
    g_nat = nat8.tile([B, F1S], F32, tag="nat8", name="g_nat")
    nc.scalar.activation(out=g_nat[:], in_=ps_h1[:], func=AF.Gelu)
    gT = singles.tile([P, F1S // P, B], F16)
    t_nat_to_T(g_nat, gT, F1S // P, B, "g")

    # mm2: h2 partial (8, 4096) = g @ w2_s + b2/8 ; AllReduce
    h2_nat = nat16.tile([B, D], F32, tag="nat16", name="h2_nat")
    for half in range(2):
        ps_h2 = psA.tile([B, HD2], F32, tag="big", name=f"ps_h2_{half}")
        for k in range(F1S // P):
            wt = wst.tile([P, HD2], F16, tag="wst", name=f"w2_t{half}_{k}")
            nc.scalar.dma_start(
                out=wt[:],
                in_=t["w2_s"][k * P:(k + 1) * P, half * HD2:(half + 1) * HD2])
            for n in range(HD2 // 512):
                nc.tensor.matmul(ps_h2[:, n * 512:(n + 1) * 512],
                                 gT[:, k, :], wt[:, n * 512:(n + 1) * 512],
                                 start=(k == 0), stop=False)
        bias_mm(ps_h2, b2_sb[:, half * HD2:(half + 1) * HD2], HD2)
        evict(h2_nat[:, half * HD2:(half + 1) * HD2], ps_h2[:])
    nc.gpsimd.dma_start(out=t["cc_mlp_in"][:], in_=h2_nat[:])
    nc.gpsimd.collective_compute(
        "AllReduce", ALU.add, replica_groups=GROUPS,
        ins=[t["cc_mlp_in"][:].opt()], outs=[t["cc_mlp_out"][:].opt()])

    # =======================================================================
    # STEP 9: x_pool = attn_out + h ; diffusion tail (replicated on all cores)
    # =======================================================================
    hug = nat16.tile([B, D], F32, tag="nat16", name="hug")
    nc.sync.dma_start(out=hug[:], in_=t["cc_mlp_out"][:])
    nc.vector.tensor_add(out=attn_nat[:], in0=attn_nat[:], in1=hug[:])
    xpT = singles.tile([P, DC, B], F16)
    t_nat_to_T(attn_nat, xpT, DC, B, "xp")

    # x0 (8, 256) = x_pool@rin_pool + cond@rin_cond + na@rin_na + rin_b
    ps_x0 = psA.tile([B, HID], F32, tag="big", name="ps_x0")
    for k in range(DC):
        wt = wst.tile([P, HID], F16, tag="wst", name=f"rp_t{k}")
        nc.scalar.dma_start(out=wt[:], in_=t["rin_pool"][k * P:(k + 1) * P, :])
        nc.tensor.matmul(ps_x0[:], xpT[:, k, :], wt[:], start=(k == 0),
                         stop=False)
    nc.tensor.matmul(ps_x0[:], condT[:], rc_sb[:], start=False, stop=False)
    nc.tensor.matmul(ps_x0[:], naT_sb[:], rna_sb[:], start=False, stop=False)
    bias_mm(ps_x0, rb_sb, HID)
    x_nat = singles.tile([B, HID], F32)
    evict(x_nat[:], ps_x0[:])

    # ---- 3 residual blocks ----
    for i in range(NBLK):
        xn = singles.tile([B, HID], F32, name=f"xn_{i}")
        layernorm_nat(x_nat[:], B, HID, xn[:], f"lnb{i}")
        xnT = singles.tile([P, HC, B], F16, name=f"xnT_{i}")
        t_nat_to_T(xn, xnT, HC, B, f"xn{i}")
        for c in range(HC):  # LN affine in T layout
            nc.vector.tensor_scalar(out=xnT[:, c, :], in0=xnT[:, c, :],
                                    scalar1=bgr_sb[:, i, c:c + 1],
                                    scalar2=bbr_sb[:, i, c:c + 1],
                                    op0=ALU.mult, op1=ALU.add)

        ps_bh = psA.tile([B, 4 * HID], F32, tag="big", name=f"ps_bh_{i}")
        for k in range(HC):
            wt = wst.tile([P, 4 * HID], F16, tag="wst", name=f"bw1_t{i}_{k}")
            nc.scalar.dma_start(out=wt[:], in_=t["blk_w1"][i, k * P:(k + 1) * P, :])
            for n in range(4 * HID // 512):
                nc.tensor.matmul(ps_bh[:, n * 512:(n + 1) * 512],
                                 xnT[:, k, :], wt[:, n * 512:(n + 1) * 512],
                                 start=(k == 0), stop=False)
        bias_mm(ps_bh, bb1_sb[:, i, :], 4 * HID)
        hb = nat8.tile([B, 4 * HID], F32, tag="nat8", name=f"hb_{i}")
        nc.scalar.activation(out=hb[:], in_=ps_bh[:], func=AF.Silu)
        hbT = singles.tile([P, 4 * HID // P, B], F16, name=f"hbT_{i}")
        t_nat_to_T(hb, hbT, 4 * HID // P, B, f"hb{i}")

        ps_bo = psA.tile([B, HID], F32, tag="big", name=f"ps_bo_{i}")
        for k in range(4 * HID // P):
            wt = wst.tile([P, HID], F16, tag="wst", name=f"bw2_t{i}_{k}")
            nc.scalar.dma_start(out=wt[:], in_=t["blk_w2"][i, k * P:(k + 1) * P, :])
            nc.tensor.matmul(ps_bo[:], hbT[:, k, :], wt[:],
                             start=(k == 0), stop=False)
        bias_mm(ps_bo, bb2_sb[:, i, :], HID)
        nc.vector.tensor_add(out=x_nat[:], in0=x_nat[:], in1=ps_bo[:])

    # ---- final: res (8, 7) = swish(x) @ out_w + out_b
    nc.scalar.activation(out=x_nat[:], in_=x_nat[:], func=AF.Silu)
    xsT = singles.tile([P, HC, B], F16)
    t_nat_to_T(x_nat, xsT, HC, B, "xs")
    ps_o = psB.tile([P, 8], F32, tag="tp8", name="ps_o")
    for k in range(HC):
        nc.tensor.matmul(ps_o[:B, :AD], xsT[:, k, :], ow_sb[:, k, :],
                         start=(k == 0), stop=(k == HC - 1))
    out_sb = singles.tile([B, AD], F32)
    nc.vector.tensor_add(out=out_sb[:], in0=ps_o[:B, :AD], in1=ob_bc[:])
    nc.sync.dma_start(out=t["res"][:], in_=out_sb[:])


_CACHED_NC = None


def _get_nc():
    global _CACHED_NC
    if _CACHED_NC is None:
        _CACHED_NC = build_program()
    return _CACHED_NC


def _prep_in_maps(inputs):
    f32 = np.float32
    f16 = np.float16
    llm_full = np.ascontiguousarray(np.asarray(inputs["llm_output"], dtype=f32))
    wq = np.asarray(inputs["wq"], f32); wk = np.asarray(inputs["wk"], f32)
    wv = np.asarray(inputs["wv"], f32); wo = np.asarray(inputs["wo"], f32)
    bq = np.asarray(inputs["bq"], f32); bv = np.asarray(inputs["bv"], f32)
    bo = np.asarray(inputs["bo"], f32)
    w1 = np.asarray(inputs["mlp_w1"], f32); b1 = np.asarray(inputs["mlp_b1"], f32)
    w2 = np.asarray(inputs["mlp_w2"], f32); b2 = np.asarray(inputs["mlp_b2"], f32)
    rin_w = np.asarray(inputs["rin_w"], f32)
    probe = np.asarray(inputs["probe"], f32).reshape(D)

    def r128(v):  # (n*128,) -> (128, n) partition-major
        return np.ascontiguousarray(v.reshape(-1, P).T)

    blk_g = np.asarray(inputs["blk_ln_g"], f32)
    blk_b = np.asarray(inputs["blk_ln_b"], f32)

    shared = {
        "bo16": (bo / NC).astype(f16).reshape(1, D),
        "ln_g_r": r128(np.asarray(inputs["ln_g"], f32)),
        "ln_b_r": r128(np.asarray(inputs["ln_b"], f32)),
        "b216": (b2 / NC).astype(f16).reshape(1, D),
        "probe_r": r128(probe).astype(f16),
        "four_w2": np.concatenate(
            [np.asarray(inputs["four_w"], f32).reshape(TD // 2, 1)] * 2),
        "phase2": np.concatenate(
            [np.full((TD // 2, 1), np.pi / 2, f32),
             np.zeros((TD // 2, 1), f32)]),
        "timeT": np.ascontiguousarray(np.asarray(inputs["time"], f32).T),
        "naT": np.ascontiguousarray(
            np.asarray(inputs["noisy_actions"], f32).T).astype(f16),
        "cond_w1": np.asarray(inputs["cond_w1"], f32).astype(f16),
        "cond_b1c": np.asarray(inputs["cond_b1"], f32).reshape(-1, 1),
        "cond_w2": np.asarray(inputs["cond_w2"], f32).astype(f16),
        "cond_b2c": np.asarray(inputs["cond_b2"], f32).reshape(-1, 1),
        "rin_cond": np.ascontiguousarray(rin_w[0:TD]).astype(f16),
        "rin_pool": np.ascontiguousarray(rin_w[TD:TD + D]).astype(f16),
        "rin_na": np.ascontiguousarray(rin_w[TD + D:]).astype(f16),
        "rb16": np.asarray(inputs["rin_b"], f32).astype(f16).reshape(1, HID),
        "blk_g_r": np.ascontiguousarray(
            blk_g.reshape(NBLK, HC, P).transpose(0, 2, 1)),
        "blk_b_r": np.ascontiguousarray(
            blk_b.reshape(NBLK, HC, P).transpose(0, 2, 1)),
        "blk_w1": np.asarray(inputs["blk_w1"], f32).astype(f16),
        "blk_b1_16": np.asarray(inputs["blk_b1"], f32).astype(f16),
        "blk_w2": np.asarray(inputs["blk_w2"], f32).astype(f16),
        "blk_b2_16": np.asarray(inputs["blk_b2"], f32).astype(f16),
        "out_w": np.asarray(inputs["out_w"], f32).astype(f16),
        "out_bc": np.asarray(inputs["out_b"], f32).reshape(1, AD),
    }

    in_maps = []
    for i in range(NC):
        hb = slice(i * DH, (i + 1) * DH)
        fb = slice(i * F1S, (i + 1) * F1S)
        m = dict(shared)
        m["llm"] = llm_full[i].astype(f16)
        m["llmT"] = np.ascontiguousarray(llm_full[i].T).astype(f16)
        m["wq_s"] = np.ascontiguousarray(wq[:, hb]).astype(f16)
        m["bq_s"] = np.ascontiguousarray(bq[hb]).reshape(1, DH)
        m["wkT_s"] = np.ascontiguousarray(wk[:, hb].T).astype(f16)
        m["wv_s"] = np.ascontiguousarray(wv[:, hb]).astype(f16)
        m["bv16"] = np.ascontiguousarray(bv[hb]).astype(f16).reshape(1, DH)
        m["wo_s"] = np.ascontiguousarray(wo[hb, :]).astype(f16)
        m["w1_s"] = np.ascontiguousarray(w1[:, fb]).astype(f16)
        m["b116"] = np.ascontiguousarray(b1[fb]).astype(f16).reshape(1, F1S)
        m["w2_s"] = np.ascontiguousarray(w2[fb, :]).astype(f16)
        in_maps.append(m)
    return in_maps


def kernel(**inputs):
    nc = _get_nc()
    in_maps = _prep_in_maps(inputs)
    r = run_bass_kernel_spmd(nc, in_maps, core_ids=list(range(NC)))
    return np.ascontiguousarray(r.results[0]["res"]).astype(np.float32)


def run_traced(**inputs):
    """Like kernel() but with NTFF tracing; returns (output, results)."""
    nc = _get_nc()
    in_maps = _prep_in_maps(inputs)
    r = run_bass_kernel_spmd(nc, in_maps, core_ids=list(range(NC)), trace=True)
    return np.ascontiguousarray(r.results[0]["res"]).astype(np.float32), r



# revision 10
# speedup vs baseline: 1.1989x; 1.1989x over previous
"""Trainium2 Bass kernel for nn_DiffusionActionHead (B=8, S=2048, D=4096).

Strategy (8 NeuronCores):
  - Data-parallel over batch for everything touching llm_output; tensor-
    parallel weight reads (core i: head-slice i of wv/wo, hidden-slice i of
    mlp_w1/mlp_w2), tiny diffusion tail replicated.
  - MAP-head attention with q_len=1 collapsed algebraically:
        scores[s,h] = llm[s,:] . U[:,h]
        pooled[h,:] = softmax(scores)[h,:] @ llm
    U = wk[:,h-block] @ q_h / sqrt(DH) is input-independent (probe/wq/bq/wk
    are all parameters), so U is folded on the host -> no wq/wk streams, no
    AllGather.  (bk shifts scores by a per-head constant -> cancels in
    softmax.)
  - 3 collectives: AllToAll(pooled f16, head<->batch), AllReduce(attn
    partial), AllReduce(x0 partial (B,256) -- the rin_w projection is folded
    through the mlp AllReduce by linearity, shrinking payload 128KB -> 8KB
    and shortening the tail).
  - Large matmuls run in fp16 (accumulation fp32); softmax/LN fp32.
  - Biases folded into the PSUM accumulations via a ones-row matmul (bias
    rows stored 512-wide per partition-row to save SBUF columns); additive
    biases of AllReduce'd partials are pre-divided by 8 on host.
  - Activations kept feature-on-partition so big matmuls stream weights in
    natural DRAM layout; llm passed in both layouts (llmT host-transposed).
  - llm streams ride the sync HWDGE ring, weight streams the scalar HWDGE
    ring; w1/w2 get deep private pools so DMA stays busy across the
    collective barriers; tail weights/rin stream through the w1 ring during
    mm2 so the tail never waits on DMA.
"""

import numpy as np
import sys

if "/opt/trn_rl_repo" not in sys.path:
    sys.path.insert(0, "/opt/trn_rl_repo")

import concourse.bass as bass
import concourse.tile as tile
from concourse import bacc, mybir
from concourse.masks import make_identity
from concourse.bass_utils import run_bass_kernel_spmd

F32 = mybir.dt.float32
F16 = mybir.dt.float16
AF = mybir.ActivationFunctionType
ALU = mybir.AluOpType

B, S, D = 8, 2048, 4096
H, AD, TD, HID, NBLK = 8, 7, 32, 256, 3
DH = D // H            # 512
NC = 8                 # cores
P = 128
SC = S // P            # 16 S-chunks
DC = D // P            # 32 D-chunks
HD2 = D // 2           # 2048 (half width -> 4-bank PSUM tiles)
F1S = 4 * D // NC      # 2048 per-core hidden cols of mlp_w1
HC = HID // P          # 2
TWO_PI = 2.0 * float(np.pi)


def _bcast(src_ap, nparts):
    """Partition-broadcast a (1, N) DRAM AP to (nparts, N)."""
    ap = src_ap
    assert ap.shape[0] == 1, ap.shape
    return bass.AP(tensor=ap.tensor, offset=ap.offset,
                   ap=[[0, nparts]] + [list(x) for x in ap.ap[1:]])


def build_program():
    nc = bacc.Bacc("TRN2", target_bir_lowering=False, debug=False,
                   num_devices=NC)
    t = {}

    def din(name, shape, dtype=F32):
        t[name] = nc.dram_tensor(name, shape, dtype, kind="ExternalInput")

    din("llm", [S, D], F16); din("llmT", [D, S], F16)
    din("u_r", [P, DC, H], F16)
    din("wv_s", [D, DH], F16); din("bv16", [1, DH], F16)
    din("wo_s", [DH, D], F16); din("bo16", [1, D], F16)        # bo/8
    din("ln_g_r", [P, DC]); din("ln_b_r", [P, DC])
    din("w1_s", [D, F1S], F16); din("b116", [1, F1S], F16)
    din("w2_s", [F1S, D], F16); din("b216", [1, D], F16)       # b2/8
    din("four_w2", [TD, 1]); din("phase2", [TD, 1])
    din("timeT", [1, B]); din("naT", [AD, B], F16)
    din("cond_w1", [TD, 2 * TD], F16); din("cond_b1c", [2 * TD, 1])
    din("cond_w2", [2 * TD, TD], F16); din("cond_b2c", [TD, 1])
    din("rin_cond8", [TD, HID], F16)           # rin_w[cond rows] / 8
    din("rp_r", [P, DC, HID], F16)             # rin_w[pooled rows] p-major
    din("rin_na8", [AD, HID], F16)             # rin_w[na rows] / 8
    din("rb16", [1, HID], F16)                 # rin_b / 8
    din("blk_g_r", [NBLK, P, HC]); din("blk_b_r", [NBLK, P, HC])
    din("bw1_r", [P, NBLK, HC, 4 * HID], F16)
    din("blk_b1_16", [NBLK, 4 * HID], F16)
    din("bw2_r", [P, NBLK, 4 * HID // P, HID], F16)
    din("blk_b2_16", [NBLK, HID], F16)
    din("out_w", [HID, AD], F16); din("out_bc", [1, AD])
    t["res"] = nc.dram_tensor("res", [B, AD], F32, kind="ExternalOutput")

    # collective bounce buffers (internal DRAM; outputs in Shared space)
    t["cc_pool_in"] = nc.dram_tensor("cc_pool_in", [H, D], F16)
    t["cc_pool_out"] = nc.dram_tensor("cc_pool_out", [B, D], F16)
    t["cc_attn_in"] = nc.dram_tensor("cc_attn_in", [B, D], F16)
    t["cc_attn_out"] = nc.dram_tensor("cc_attn_out", [B, D], F16,
                                      addr_space="Shared")
    t["cc_z_in"] = nc.dram_tensor("cc_z_in", [B, HID], F32)
    t["cc_z_out"] = nc.dram_tensor("cc_z_out", [B, HID], F32,
                                   addr_space="Shared")

    with tile.TileContext(nc) as tc:
        import contextlib
        with contextlib.ExitStack() as ctx:
            _build(nc, tc, t, ctx)
    nc.finalize()
    return nc


def _build(nc, tc, t, ctx):
    GROUPS = [list(range(NC))]

    singles = ctx.enter_context(tc.tile_pool(name="singles", bufs=1))
    llm_pool = ctx.enter_context(tc.tile_pool(name="llm_pool", bufs=6))
    llmT_pool = ctx.enter_context(tc.tile_pool(name="llmT_pool", bufs=6))
    wvwo = ctx.enter_context(tc.tile_pool(name="wvwo", bufs=5))
    w1p = ctx.enter_context(tc.tile_pool(name="w1p", bufs=10))
    w2p = ctx.enter_context(tc.tile_pool(name="w2p", bufs=5))
    nat16 = ctx.enter_context(tc.tile_pool(name="nat16", bufs=2))
    nat8 = ctx.enter_context(tc.tile_pool(name="nat8", bufs=2))
    psA = ctx.enter_context(tc.tile_pool(name="psA", bufs=1, space="PSUM"))
    psB = ctx.enter_context(tc.tile_pool(name="psB", bufs=2, space="PSUM"))
    psC = ctx.enter_context(tc.tile_pool(name="psC", bufs=2, space="PSUM"))

    ident = singles.tile([P, P], F32)
    make_identity(nc, ident)
    ident16 = singles.tile([P, P], F16)
    nc.vector.tensor_copy(out=ident16[:], in_=ident[:])
    eps_sb = singles.tile([P, 1], F32)
    nc.vector.memset(eps_sb[:], 1e-5)
    ones8 = singles.tile([1, 8], F16)
    nc.vector.memset(ones8[:], 1.0)

    def evict(dst, src):
        nc.vector.tensor_copy(out=dst, in_=src)

    def t_nat_to_T(src_nat, dst_T, nchunks, npart, uid):
        """(npart, nchunks*128) sbuf -> (128, nchunks, npart) sbuf via PE."""
        idn = ident16 if src_nat.dtype == F16 else ident
        for c in range(nchunks):
            ps = psB.tile([P, 8], src_nat.dtype, tag="tp8", name=f"tp_{uid}_{c}")
            nc.tensor.transpose(ps[:, :npart], src_nat[:, c * P:(c + 1) * P],
                                idn[:npart, :npart])
            evict(dst_T[:, c, :], ps[:, :npart])

    def bias_mm(ps, bias_row, n_total, stop=True):
        """Add a (1, n_total) f16 bias row into psum (8, n_total) via ones-row
        matmuls, 512 cols per matmul (moving-dim limit)."""
        nch = (n_total + 511) // 512
        for n in range(nch):
            w = min(512, n_total - n * 512)
            nc.tensor.matmul(ps[:, n * 512:n * 512 + w], ones8[:, :B],
                             bias_row[:, n * 512:n * 512 + w],
                             start=False, stop=(stop and n == nch - 1))

    def layernorm_nat(x_nat, npart, n, y_nat, uid):
        """y = (x - mean) / sqrt(var + eps) over the free dim of (npart, n)."""
        nsub = max(1, n // 512)
        st = nat8.tile([npart, nsub, nc.vector.BN_STATS_DIM], F32, tag="lnst",
                       name=f"lnst_{uid}")
        xg = x_nat.rearrange("p (a b) -> p a b", a=nsub)
        for g in range(nsub):
            nc.vector.bn_stats(out=st[:, g, :], in_=xg[:, g, :])
        mv = nat8.tile([npart, nc.vector.BN_AGGR_DIM], F32, tag="lnmv",
                       name=f"lnmv_{uid}")
        nc.vector.bn_aggr(out=mv[:], in_=st[:])
        std = nat8.tile([npart, 1], F32, tag="lnsd", name=f"lnsd_{uid}")
        nc.scalar.activation(out=std[:], in_=mv[:, 1:2], func=AF.Sqrt,
                             bias=eps_sb[:npart, :])
        nc.vector.reciprocal(out=std[:], in_=std[:])
        nc.vector.tensor_scalar(out=y_nat, in0=x_nat, scalar1=mv[:, 0:1],
                                scalar2=std[:], op0=ALU.subtract, op1=ALU.mult)

    # =======================================================================
    # STEP 0: constants, bias rows — prefetched early on queues that are
    # otherwise idle so later phases never wait on them.
    # =======================================================================
    u_sb = singles.tile([P, DC, H], F16)
    nc.sync.dma_start(out=u_sb[:], in_=t["u_r"][:])
    bv_sb = singles.tile([1, DH], F16)
    nc.gpsimd.dma_start(out=bv_sb[:], in_=t["bv16"][:])
    bo_sb = singles.tile([1, D], F16)
    nc.gpsimd.dma_start(out=bo_sb[:], in_=t["bo16"][:])
    b1_sb = singles.tile([1, F1S], F16)
    nc.gpsimd.dma_start(out=b1_sb[:], in_=t["b116"][:])
    b2_sb = singles.tile([1, D], F16)
    nc.gpsimd.dma_start(out=b2_sb[:], in_=t["b216"][:])
    rb_sb = singles.tile([1, HID], F16)
    nc.gpsimd.dma_start(out=rb_sb[:], in_=t["rb16"][:])
    bb1_sb = singles.tile([1, NBLK, 4 * HID], F16)
    nc.gpsimd.dma_start(out=bb1_sb[:], in_=t["blk_b1_16"][:].rearrange("n f -> (n f)")[None, :])
    bb2_sb = singles.tile([1, NBLK, HID], F16)
    nc.gpsimd.dma_start(out=bb2_sb[:], in_=t["blk_b2_16"][:].rearrange("n f -> (n f)")[None, :])
    lng_sb = singles.tile([P, DC], F32)
    nc.sync.dma_start(out=lng_sb[:], in_=t["ln_g_r"][:])
    lnb_sb = singles.tile([P, DC], F32)
    nc.sync.dma_start(out=lnb_sb[:], in_=t["ln_b_r"][:])
    bgr_sb = singles.tile([P, NBLK, HC], F32)
    nc.sync.dma_start(out=bgr_sb[:],
                      in_=t["blk_g_r"][:].rearrange("n p c -> p n c"))
    bbr_sb = singles.tile([P, NBLK, HC], F32)
    nc.sync.dma_start(out=bbr_sb[:],
                      in_=t["blk_b_r"][:].rearrange("n p c -> p n c"))
    rc_sb = singles.tile([TD, HID], F16)
    nc.gpsimd.dma_start(out=rc_sb[:], in_=t["rin_cond8"][:])
    rna_sb = singles.tile([AD, HID], F16)
    nc.gpsimd.dma_start(out=rna_sb[:], in_=t["rin_na8"][:])
    naT_sb = singles.tile([AD, B], F16)
    nc.sync.dma_start(out=naT_sb[:], in_=t["naT"][:])
    ow_sb = singles.tile([P, HC, AD], F16)
    nc.sync.dma_start(out=ow_sb[:],
                      in_=t["out_w"][:].rearrange("(c p) a -> p c a", p=P))
    ob_bc = singles.tile([B, AD], F32)
    nc.gpsimd.dma_start(out=ob_bc[:], in_=_bcast(t["out_bc"][:], B))

    # ---- cond path (fourier + tiny mlp) — independent of everything else.
    fw_sb = singles.tile([TD, 1], F32)
    nc.sync.dma_start(out=fw_sb[:], in_=t["four_w2"][:])
    ph_sb = singles.tile([TD, 1], F32)
    nc.sync.dma_start(out=ph_sb[:], in_=t["phase2"][:])
    tb32 = singles.tile([TD, B], F32)
    nc.gpsimd.dma_start(out=tb32[:], in_=_bcast(t["timeT"][:], TD))
    fu = singles.tile([TD, B], F32)
    nc.vector.tensor_scalar_mul(out=fu[:], in0=tb32[:], scalar1=fw_sb[:])
    # exact range reduction: sin/cos have period 1 in fu, so subtract the
    # integer part via an f32->i32->f32 round-trip (|fu| < ~64 here).
    fi = singles.tile([TD, B], mybir.dt.int32)
    nc.vector.tensor_copy(out=fi[:], in_=fu[:])
    fif = singles.tile([TD, B], F32)
    nc.vector.tensor_copy(out=fif[:], in_=fi[:])
    nc.vector.tensor_sub(out=fu[:], in0=fu[:], in1=fif[:])
    ffT = singles.tile([TD, B], F16)
    nc.scalar.activation(out=ffT[:], in_=fu[:], func=AF.Sin,
                         scale=TWO_PI, bias=ph_sb[:])
    cw1_sb = singles.tile([TD, 2 * TD], F16)
    nc.scalar.dma_start(out=cw1_sb[:], in_=t["cond_w1"][:])
    cb1_sb = singles.tile([2 * TD, 1], F32)
    nc.sync.dma_start(out=cb1_sb[:], in_=t["cond_b1c"][:])
    cw2_sb = singles.tile([2 * TD, TD], F16)
    nc.scalar.dma_start(out=cw2_sb[:], in_=t["cond_w2"][:])
    cb2_sb = singles.tile([TD, 1], F32)
    nc.sync.dma_start(out=cb2_sb[:], in_=t["cond_b2c"][:])
    ps_c1 = psB.tile([P, 8], F32, tag="tp8", name="ps_c1")
    nc.tensor.matmul(ps_c1[:2 * TD, :B], cw1_sb[:], ffT[:], start=True, stop=True)
    c1 = singles.tile([2 * TD, B], F16)
    nc.scalar.activation(out=c1[:], in_=ps_c1[:2 * TD, :B], func=AF.Silu,
                         bias=cb1_sb[:])
    ps_c2 = psB.tile([P, 8], F32, tag="tp8", name="ps_c2")
    nc.tensor.matmul(ps_c2[:TD, :B], cw2_sb[:], c1[:], start=True, stop=True)
    condT = singles.tile([TD, B], F16)
    nc.scalar.activation(out=condT[:], in_=ps_c2[:TD, :B], func=AF.Identity,
                         bias=cb2_sb[:])

    # =======================================================================
    # STEP 1: scoresT (8, 2048) = U.T @ llmT  (fp16 inputs, fp32 accum)
    # =======================================================================
    ps_sc = psA.tile([H, S], F32, tag="big", name="ps_sc")
    for k in range(DC):
        lt = llmT_pool.tile([P, S], F16, tag="llmT", name=f"llmT_t{k}")
        nc.sync.dma_start(out=lt[:], in_=t["llmT"][k * P:(k + 1) * P, :])
        for n in range(S // 512):
            nc.tensor.matmul(ps_sc[:, n * 512:(n + 1) * 512],
                             u_sb[:, k, :], lt[:, n * 512:(n + 1) * 512],
                             start=(k == 0), stop=(k == DC - 1))

    # =======================================================================
    # STEP 2: softmax over S. Max-subtraction skipped deliberately: softmax
    # is shift-invariant and |scores| < ~1 here, so exp() is perfectly
    # conditioned; the result is mathematically identical.
    # =======================================================================
    p_nat = nat8.tile([H, S], F32, tag="nat8", name="p_nat")
    nc.scalar.activation(out=p_nat[:], in_=ps_sc[:], func=AF.Exp)
    den = singles.tile([H, 1], F32)
    nc.vector.reduce_sum(out=den[:], in_=p_nat[:], axis=mybir.AxisListType.X)
    nc.vector.reciprocal(out=den[:], in_=den[:])
    nc.vector.tensor_scalar_mul(out=p_nat[:], in0=p_nat[:], scalar1=den[:])
    pT = singles.tile([P, SC, H], F16)
    t_nat_to_T(p_nat, pT, SC, H, "p")

    # =======================================================================
    # STEP 3: pooled (8, 4096) = pT.T @ llm ; AllToAll (head <-> batch)
    # =======================================================================
    pooled_nat = nat16.tile([H, D], F16, tag="nat16", name="pooled_nat")
    for half in range(2):
        ps_p = psA.tile([H, HD2], F32, tag="big", name=f"ps_pool_{half}")
        for s in range(SC):
            lt = llm_pool.tile([P, HD2], F16, tag="llm", name=f"llm_t{half}_{s}")
            nc.sync.dma_start(
                out=lt[:],
                in_=t["llm"][s * P:(s + 1) * P, half * HD2:(half + 1) * HD2])
            for n in range(HD2 // 512):
                nc.tensor.matmul(ps_p[:, n * 512:(n + 1) * 512],
                                 pT[:, s, :], lt[:, n * 512:(n + 1) * 512],
                                 start=(s == 0), stop=(s == SC - 1))
        evict(pooled_nat[:, half * HD2:(half + 1) * HD2], ps_p[:])

    nc.sync.dma_start(out=t["cc_pool_in"][:], in_=pooled_nat[:])
    nc.gpsimd.collective_compute(
        "AllToAll", ALU.bypass, replica_groups=GROUPS,
        ins=[t["cc_pool_in"][:].opt()], outs=[t["cc_pool_out"][:].opt()])

    # =======================================================================
    # STEP 4: ctx for this core's head, all batches: (8, 512) = poolh@wv + bv
    # =======================================================================
    poolh_nat = nat16.tile([B, D], F16, tag="nat16", name="poolh_nat")
    nc.sync.dma_start(out=poolh_nat[:], in_=t["cc_pool_out"][:])
    poolhT = singles.tile([P, DC, B], F16)
    t_nat_to_T(poolh_nat, poolhT, DC, B, "ph")

    ps_cx = psC.tile([B, DH], F32, tag="vec", name="ps_cx")
    wv_r = t["wv_s"].rearrange("(c p) n -> p c n", p=P)
    for g in range(8):
        wt = wvwo.tile([P, 4, DH], F16, tag="wst", name=f"wv_g{g}")
        nc.scalar.dma_start(out=wt[:], in_=wv_r[:, 4 * g:4 * g + 4, :])
        for j in range(4):
            k = 4 * g + j
            nc.tensor.matmul(ps_cx[:], poolhT[:, k, :], wt[:, j, :],
                             start=(k == 0), stop=False)
    bias_mm(ps_cx, bv_sb, DH)
    ctx_nat = nat8.tile([B, DH], F32, tag="nat8", name="ctx_nat")
    evict(ctx_nat[:], ps_cx[:])
    ctxT = singles.tile([P, DH // P, B], F16)
    t_nat_to_T(ctx_nat, ctxT, DH // P, B, "cx")

    # =======================================================================
    # STEP 5: attn partial (8, 4096) = ctx @ wo_s + bo/8 ; AllReduce
    # =======================================================================
    attn_part = nat16.tile([B, D], F16, tag="nat16", name="attn_part")
    for half in range(2):
        ps_a = psA.tile([B, HD2], F32, tag="big", name=f"ps_attn_{half}")
        for k in range(DH // P):
            wt = wvwo.tile([P, HD2], F16, tag="wst", name=f"wo_t{half}_{k}")
            nc.scalar.dma_start(
                out=wt[:],
                in_=t["wo_s"][k * P:(k + 1) * P, half * HD2:(half + 1) * HD2])
            for n in range(HD2 // 512):
                nc.tensor.matmul(ps_a[:, n * 512:(n + 1) * 512],
                                 ctxT[:, k, :], wt[:, n * 512:(n + 1) * 512],
                                 start=(k == 0), stop=False)
        bias_mm(ps_a, bo_sb[:, half * HD2:(half + 1) * HD2], HD2)
        evict(attn_part[:, half * HD2:(half + 1) * HD2], ps_a[:])
    nc.sync.dma_start(out=t["cc_attn_in"][:], in_=attn_part[:])
    nc.gpsimd.collective_compute(
        "AllReduce", ALU.add, replica_groups=GROUPS,
        ins=[t["cc_attn_in"][:].opt()], outs=[t["cc_attn_out"][:].opt()])

    # =======================================================================
    # STEP 6: y = LN(attn_out)*g+b ; mlp partial h2 = gelu(y@w1+b1)@w2 + b2/8
    # =======================================================================
    attn_nat = singles.tile([B, D], F16)  # persists (residual)
    nc.sync.dma_start(out=attn_nat[:], in_=t["cc_attn_out"][:])
    # attn_out/8 staged into sum_pre now (cheap, off critical path); the mm2
    # partials are added in-place per half below.
    sum_pre = nat16.tile([B, D], F16, tag="nat16", name="sum_pre")
    nc.vector.tensor_scalar_mul(out=sum_pre[:], in0=attn_nat[:], scalar1=0.125)

    y_nat = nat16.tile([B, D], F16, tag="nat16", name="y_nat")
    layernorm_nat(attn_nat[:], B, D, y_nat[:], "ln0")
    yT = singles.tile([P, DC, B], F16)
    t_nat_to_T(y_nat, yT, DC, B, "y")
    # LN affine in T layout (gamma/beta become per-partition scalars)
    for c in range(DC):
        nc.vector.tensor_scalar(out=yT[:, c, :], in0=yT[:, c, :],
                                scalar1=lng_sb[:, c:c + 1],
                                scalar2=lnb_sb[:, c:c + 1],
                                op0=ALU.mult, op1=ALU.add)

    # mm1: h1 (8, 2048) = y @ w1_s + b1 ; exact gelu straight off PSUM
    ps_h1 = psA.tile([B, F1S], F32, tag="big", name="ps_h1")
    for k in range(DC):
        wt = w1p.tile([P, F1S], F16, tag="w1", name=f"w1_t{k}")
        nc.scalar.dma_start(out=wt[:], in_=t["w1_s"][k * P:(k + 1) * P, :])
        for n in range(F1S // 512):
            nc.tensor.matmul(ps_h1[:, n * 512:(n + 1) * 512],
                             yT[:, k, :], wt[:, n * 512:(n + 1) * 512],
                             start=(k == 0), stop=False)
    bias_mm(ps_h1, b1_sb, F1S)
    g_nat = nat8.tile([B, F1S], F32, tag="nat8", name="g_nat")
    nc.scalar.activation(out=g_nat[:], in_=ps_h1[:], func=AF.Gelu)
    gT = singles.tile([P, F1S // P, B], F16)
    t_nat_to_T(g_nat, gT, F1S // P, B, "g")

    # mm2: h2 partial (8, 4096) = g @ w2_s + b2/8, accumulated into sum_pre
    for half in range(2):
        ps_h2 = psA.tile([B, HD2], F32, tag="big", name=f"ps_h2_{half}")
        for k in range(F1S // P):
            wt = w2p.tile([P, HD2], F16, tag="w2", name=f"w2_t{half}_{k}")
            nc.scalar.dma_start(
                out=wt[:],
                in_=t["w2_s"][k * P:(k + 1) * P, half * HD2:(half + 1) * HD2])
            for n in range(HD2 // 512):
                nc.tensor.matmul(ps_h2[:, n * 512:(n + 1) * 512],
                                 gT[:, k, :], wt[:, n * 512:(n + 1) * 512],
                                 start=(k == 0), stop=False)
        bias_mm(ps_h2, b2_sb[:, half * HD2:(half + 1) * HD2], HD2)
        nc.vector.tensor_add(
            out=sum_pre[:, half * HD2:(half + 1) * HD2],
            in0=sum_pre[:, half * HD2:(half + 1) * HD2], in1=ps_h2[:])

    spT = singles.tile([P, DC, B], F16)
    t_nat_to_T(sum_pre, spT, DC, B, "sp")

    # z (8, 256) = sum_pre@rp + cond@(rc/8) + na@(rna/8) + rb/8 ; AllReduce.
    # rp streams through the w1 ring (slots are free again during mm2).
    rp_tiles = []
    for g in range(4):
        rpt = w1p.tile([P, 8, HID], F16, tag="w1", name=f"rp_g{g}")
        nc.scalar.dma_start(out=rpt[:], in_=t["rp_r"][:, 8 * g:8 * (g + 1), :])
        rp_tiles.append(rpt)
    ps_z = psC.tile([B, HID], F32, tag="vec", name="ps_z")
    for k in range(DC):
        nc.tensor.matmul(ps_z[:], spT[:, k, :], rp_tiles[k // 8][:, k % 8, :],
                         start=(k == 0), stop=False)
    nc.tensor.matmul(ps_z[:], condT[:], rc_sb[:], start=False, stop=False)
    nc.tensor.matmul(ps_z[:], naT_sb[:], rna_sb[:], start=False, stop=False)
    bias_mm(ps_z, rb_sb, HID)
    z_nat = nat8.tile([B, HID], F32, tag="nat8", name="z_nat")
    evict(z_nat[:], ps_z[:])
    nc.sync.dma_start(out=t["cc_z_in"][:], in_=z_nat[:])
    nc.gpsimd.collective_compute(
        "AllReduce", ALU.add, replica_groups=GROUPS,
        ins=[t["cc_z_in"][:].opt()], outs=[t["cc_z_out"][:].opt()])

    # tail block weights stream through the w1 ring during mm2/z as well
    bw1_tiles, bw2_tiles = [], []
    for i in range(NBLK):
        bt1 = w1p.tile([P, HC, 4 * HID], F16, tag="w1", name=f"bw1_{i}")
        nc.scalar.dma_start(out=bt1[:], in_=t["bw1_r"][:, i, :, :])
        bw1_tiles.append(bt1)
        bt2 = w1p.tile([P, 4 * HID // P, HID], F16, tag="w1", name=f"bw2_{i}")
        nc.scalar.dma_start(out=bt2[:], in_=t["bw2_r"][:, i, :, :])
        bw2_tiles.append(bt2)

    # =======================================================================
    # STEP 7: diffusion tail (replicated on all cores)
    # =======================================================================
    x_nat = singles.tile([B, HID], F32)
    nc.sync.dma_start(out=x_nat[:], in_=t["cc_z_out"][:])

    # ---- 3 residual blocks ----
    for i in range(NBLK):
        xn = singles.tile([B, HID], F32, name=f"xn_{i}")
        layernorm_nat(x_nat[:], B, HID, xn[:], f"lnb{i}")
        xnT = singles.tile([P, HC, B], F16, name=f"xnT_{i}")
        t_nat_to_T(xn, xnT, HC, B, f"xn{i}")
        for c in range(HC):  # LN affine in T layout
            nc.vector.tensor_scalar(out=xnT[:, c, :], in0=xnT[:, c, :],
                                    scalar1=bgr_sb[:, i, c:c + 1],
                                    scalar2=bbr_sb[:, i, c:c + 1],
                                    op0=ALU.mult, op1=ALU.add)

        ps_bh = psA.tile([B, 4 * HID], F32, tag="big", name=f"ps_bh_{i}")
        for k in range(HC):
            for n in range(4 * HID // 512):
                nc.tensor.matmul(ps_bh[:, n * 512:(n + 1) * 512],
                                 xnT[:, k, :],
                                 bw1_tiles[i][:, k, n * 512:(n + 1) * 512],
                                 start=(k == 0), stop=False)
        bias_mm(ps_bh, bb1_sb[:, i, :], 4 * HID)
        hb = nat8.tile([B, 4 * HID], F32, tag="nat8", name=f"hb_{i}")
        nc.scalar.activation(out=hb[:], in_=ps_bh[:], func=AF.Silu)
        hbT = singles.tile([P, 4 * HID // P, B], F16, name=f"hbT_{i}")
        t_nat_to_T(hb, hbT, 4 * HID // P, B, f"hb{i}")

        ps_bo = psC.tile([B, HID], F32, tag="vec", name=f"ps_bo_{i}")
        for k in range(4 * HID // P):
            nc.tensor.matmul(ps_bo[:], hbT[:, k, :], bw2_tiles[i][:, k, :],
                             start=(k == 0), stop=False)
        bias_mm(ps_bo, bb2_sb[:, i, :], HID)
        nc.vector.tensor_add(out=x_nat[:], in0=x_nat[:], in1=ps_bo[:])

    # ---- final: res (8, 7) = swish(x) @ out_w + out_b
    nc.scalar.activation(out=x_nat[:], in_=x_nat[:], func=AF.Silu)
    xsT = singles.tile([P, HC, B], F16)
    t_nat_to_T(x_nat, xsT, HC, B, "xs")
    ps_o = psB.tile([P, 8], F32, tag="tp8", name="ps_o")
    for k in range(HC):
        nc.tensor.matmul(ps_o[:B, :AD], xsT[:, k, :], ow_sb[:, k, :],
                         start=(k == 0), stop=(k == HC - 1))
    out_sb = singles.tile([B, AD], F32)
    nc.vector.tensor_add(out=out_sb[:], in0=ps_o[:B, :AD], in1=ob_bc[:])
    nc.sync.dma_start(out=t["res"][:], in_=out_sb[:])


_CACHED_NC = None


def _get_nc():
    global _CACHED_NC
    if _CACHED_NC is None:
        _CACHED_NC = build_program()
    return _CACHED_NC


def _prep_in_maps(inputs):
    f32 = np.float32
    f16 = np.float16
    llm_full = np.ascontiguousarray(np.asarray(inputs["llm_output"], dtype=f32))
    wq = np.asarray(inputs["wq"], f32); wk = np.asarray(inputs["wk"], f32)
    wv = np.asarray(inputs["wv"], f32); wo = np.asarray(inputs["wo"], f32)
    bq = np.asarray(inputs["bq"], f32); bv = np.asarray(inputs["bv"], f32)
    bo = np.asarray(inputs["bo"], f32)
    w1 = np.asarray(inputs["mlp_w1"], f32); b1 = np.asarray(inputs["mlp_b1"], f32)
    w2 = np.asarray(inputs["mlp_w2"], f32); b2 = np.asarray(inputs["mlp_b2"], f32)
    rin_w = np.asarray(inputs["rin_w"], f32)
    probe = np.asarray(inputs["probe"], f32).reshape(D)

    # U = wk[:, hs] @ q[hs] / sqrt(DH) is a pure function of parameters
    # (the probe attention query is input-independent) -> folded here.
    q = probe @ wq + bq                       # (D,)
    U = np.empty((D, H), f32)
    for h in range(H):
        hs = slice(h * DH, (h + 1) * DH)
        U[:, h] = wk[:, hs] @ q[hs]
    U *= 1.0 / np.sqrt(DH)

    def r128(v):  # (n*128,) -> (128, n) partition-major
        return np.ascontiguousarray(v.reshape(-1, P).T)

    blk_g = np.asarray(inputs["blk_ln_g"], f32)
    blk_b = np.asarray(inputs["blk_ln_b"], f32)
    blk_w1 = np.asarray(inputs["blk_w1"], f32)   # (NBLK, HID, 4*HID)
    blk_w2 = np.asarray(inputs["blk_w2"], f32)   # (NBLK, 4*HID, HID)

    shared = {
        "u_r": np.ascontiguousarray(
            U.reshape(DC, P, H).transpose(1, 0, 2)).astype(f16),
        "bo16": (bo / NC).astype(f16).reshape(1, D),
        "ln_g_r": r128(np.asarray(inputs["ln_g"], f32)),
        "ln_b_r": r128(np.asarray(inputs["ln_b"], f32)),
        "b216": (b2 / NC).astype(f16).reshape(1, D),
        "four_w2": np.concatenate(
            [np.asarray(inputs["four_w"], f32).reshape(TD // 2, 1)] * 2),
        "phase2": np.concatenate(
            [np.full((TD // 2, 1), np.pi / 2, f32),
             np.zeros((TD // 2, 1), f32)]),
        "timeT": np.ascontiguousarray(np.asarray(inputs["time"], f32).T),
        "naT": np.ascontiguousarray(
            np.asarray(inputs["noisy_actions"], f32).T).astype(f16),
        "cond_w1": np.asarray(inputs["cond_w1"], f32).astype(f16),
        "cond_b1c": np.asarray(inputs["cond_b1"], f32).reshape(-1, 1),
        "cond_w2": np.asarray(inputs["cond_w2"], f32).astype(f16),
        "cond_b2c": np.asarray(inputs["cond_b2"], f32).reshape(-1, 1),
        "rin_cond8": (np.ascontiguousarray(rin_w[0:TD]) / NC).astype(f16),
        "rp_r": np.ascontiguousarray(
            rin_w[TD:TD + D].reshape(DC, P, HID).transpose(1, 0, 2)
        ).astype(f16),
        "rin_na8": (np.ascontiguousarray(rin_w[TD + D:]) / NC).astype(f16),
        "rb16": (np.asarray(inputs["rin_b"], f32) / NC
                 ).astype(f16).reshape(1, HID),
        "blk_g_r": np.ascontiguousarray(
            blk_g.reshape(NBLK, HC, P).transpose(0, 2, 1)),
        "blk_b_r": np.ascontiguousarray(
            blk_b.reshape(NBLK, HC, P).transpose(0, 2, 1)),
        "bw1_r": np.ascontiguousarray(
            blk_w1.reshape(NBLK, HC, P, 4 * HID).transpose(2, 0, 1, 3)
        ).astype(f16),
        "blk_b1_16": np.asarray(inputs["blk_b1"], f32).astype(f16),
        "bw2_r": np.ascontiguousarray(
            blk_w2.reshape(NBLK, 4 * HID // P, P, HID).transpose(2, 0, 1, 3)
        ).astype(f16),
        "blk_b2_16": np.asarray(inputs["blk_b2"], f32).astype(f16),
        "out_w": np.asarray(inputs["out_w"], f32).astype(f16),
        "out_bc": np.asarray(inputs["out_b"], f32).reshape(1, AD),
    }

    in_maps = []
    for i in range(NC):
        hb = slice(i * DH, (i + 1) * DH)
        fb = slice(i * F1S, (i + 1) * F1S)
        m = dict(shared)
        m["llm"] = llm_full[i].astype(f16)
        m["llmT"] = np.ascontiguousarray(llm_full[i].T).astype(f16)
        m["wv_s"] = np.ascontiguousarray(wv[:, hb]).astype(f16)
        m["bv16"] = np.ascontiguousarray(bv[hb]).astype(f16).reshape(1, DH)
        m["wo_s"] = np.ascontiguousarray(wo[hb, :]).astype(f16)
        m["w1_s"] = np.ascontiguousarray(w1[:, fb]).astype(f16)
        m["b116"] = np.ascontiguousarray(b1[fb]).astype(f16).reshape(1, F1S)
        m["w2_s"] = np.ascontiguousarray(w2[fb, :]).astype(f16)
        in_maps.append(m)
    return in_maps


def kernel(**inputs):
    nc = _get_nc()
    in_maps = _prep_in_maps(inputs)
    r = run_bass_kernel_spmd(nc, in_maps, core_ids=list(range(NC)))
    return np.ascontiguousarray(r.results[0]["res"]).astype(np.float32)


def run_traced(**inputs):
    """Like kernel() but with NTFF tracing; returns (output, results)."""
    nc = _get_nc()
    in_maps = _prep_in_maps(inputs)
    r = run_bass_kernel_spmd(nc, in_maps, core_ids=list(range(NC)), trace=True)
    return np.ascontiguousarray(r.results[0]["res"]).astype(np.float32), r


# revision 11
# speedup vs baseline: 1.2412x; 1.0353x over previous
"""Trainium2 Bass kernel for nn_DiffusionActionHead (B=8, S=2048, D=4096).

Strategy (8 NeuronCores):
  - Data-parallel over batch for everything touching llm_output; tensor-
    parallel weight reads (core i: head-slice i of wv/wo, hidden-slice i of
    mlp_w1/mlp_w2), tiny diffusion tail replicated.
  - MAP-head attention with q_len=1 collapsed algebraically:
        scores[s,h] = llm[s,:] . U[:,h]
        pooled[h,:] = softmax(scores)[h,:] @ llm
    U = wk[:,h-block] @ q_h / sqrt(DH) is input-independent (probe/wq/bq/wk
    are all parameters), so U is folded on the host -> no wq/wk streams, no
    AllGather.  (bk shifts scores by a per-head constant -> cancels in
    softmax.)
  - 4 collectives: 2x AllToAll (pooled f16, split by D-half so the first
    A2A overlaps the second half's matmuls), AllReduce(attn partial f16),
    AllReduce(x0 partial (B,256) -- the rin_w projection is folded through
    the mlp AllReduce by linearity, shrinking payload 128KB -> 8KB).
  - Large matmuls run in fp16 (accumulation fp32); softmax/LN stats fp32.
  - LN affine (gamma/beta) is folded into w1/b1 (and blk_w1/blk_b1) on the
    host: LN(x)*g+b @ W == LN(x) @ (g*W) + (b@W + bias).
  - 1/sqrt(var+eps) computed on VectorE via Quake bit-trick + 2 Newton
    iterations -- avoids ACT Sqrt table swaps (1.3us each) in the tail.
  - Biases folded into the PSUM accumulations via a ones-row matmul;
    additive biases of AllReduce'd partials pre-divided by 8 on host.
  - ALL large streams (llmT, llm, wv, wo, w1, w2, rin_pool, blk weights)
    share ONE 28-slot ring pool: slots freed by the attention phase are
    immediately reused for weight prefetch, so DMA never idles across the
    collective barriers.  llm streams ride the sync HWDGE ring, weights the
    scalar HWDGE ring.
"""

import numpy as np
import sys

if "/opt/trn_rl_repo" not in sys.path:
    sys.path.insert(0, "/opt/trn_rl_repo")

import concourse.bass as bass
import concourse.tile as tile
from concourse import bacc, mybir
from concourse.masks import make_identity
from concourse.bass_utils import run_bass_kernel_spmd

F32 = mybir.dt.float32
F16 = mybir.dt.float16
I32 = mybir.dt.int32
AF = mybir.ActivationFunctionType
ALU = mybir.AluOpType

B, S, D = 8, 2048, 4096
H, AD, TD, HID, NBLK = 8, 7, 32, 256, 3
DH = D // H            # 512
NC = 8                 # cores
P = 128
SC = S // P            # 16 S-chunks
DC = D // P            # 32 D-chunks
HD2 = D // 2           # 2048 (half width -> 4-bank PSUM tiles)
F1S = 4 * D // NC      # 2048 per-core hidden cols of mlp_w1
HC = HID // P          # 2
TWO_PI = 2.0 * float(np.pi)


def _bcast(src_ap, nparts):
    """Partition-broadcast a (1, N) DRAM AP to (nparts, N)."""
    ap = src_ap
    assert ap.shape[0] == 1, ap.shape
    return bass.AP(tensor=ap.tensor, offset=ap.offset,
                   ap=[[0, nparts]] + [list(x) for x in ap.ap[1:]])


def build_program():
    nc = bacc.Bacc("TRN2", target_bir_lowering=False, debug=False,
                   num_devices=NC)
    t = {}

    def din(name, shape, dtype=F32):
        t[name] = nc.dram_tensor(name, shape, dtype, kind="ExternalInput")

    din("llm", [S, D], F16); din("llmT", [D, S], F16)
    din("u_r", [P, DC, H], F16)
    din("wv_s", [D, DH], F16); din("bv16", [1, DH], F16)
    din("wo_s", [DH, D], F16); din("bo16", [1, D], F16)        # bo/8
    din("w1_s", [D, F1S], F16); din("b116", [1, F1S], F16)     # g-folded
    din("w2_s", [F1S, D], F16); din("b216", [1, D], F16)       # b2/8
    din("four_w2", [TD, 1]); din("phase2", [TD, 1])
    din("timeT", [1, B]); din("naT", [AD, B], F16)
    din("cond_w1", [TD, 2 * TD], F16); din("cond_b1c", [2 * TD, 1])
    din("cond_w2", [2 * TD, TD], F16); din("cond_b2c", [TD, 1])
    din("rin_cond8", [TD, HID], F16)           # rin_w[cond rows] / 8
    din("rp_r", [P, DC, HID], F16)             # rin_w[pooled rows] p-major
    din("rin_na8", [AD, HID], F16)             # rin_w[na rows] / 8
    din("rb16", [1, HID], F16)                 # rin_b / 8
    din("bw1_r", [P, NBLK, HC, 4 * HID], F16)  # g-folded
    din("blk_b1_16", [NBLK, 4 * HID], F16)     # b-folded
    din("bw2_r", [P, NBLK, 4 * HID // P, HID], F16)
    din("blk_b2_16", [NBLK, HID], F16)
    din("out_w", [HID, AD], F16); din("out_bc", [1, AD])
    t["res"] = nc.dram_tensor("res", [B, AD], F32, kind="ExternalOutput")

    # collective bounce buffers (internal DRAM; outputs in Shared space)
    for half in range(2):
        t[f"cc_pool_in{half}"] = nc.dram_tensor(
            f"cc_pool_in{half}", [H, HD2], F16)
        t[f"cc_pool_out{half}"] = nc.dram_tensor(
            f"cc_pool_out{half}", [B, HD2], F16)
    t["cc_attn_in"] = nc.dram_tensor("cc_attn_in", [B, D], F16)
    t["cc_attn_out"] = nc.dram_tensor("cc_attn_out", [B, D], F16,
                                      addr_space="Shared")
    t["cc_z_in"] = nc.dram_tensor("cc_z_in", [B, HID], F32)
    t["cc_z_out"] = nc.dram_tensor("cc_z_out", [B, HID], F32,
                                   addr_space="Shared")

    with tile.TileContext(nc) as tc:
        import contextlib
        with contextlib.ExitStack() as ctx:
            _build(nc, tc, t, ctx)
    nc.finalize()
    return nc


def _build(nc, tc, t, ctx):
    GROUPS = [list(range(NC))]

    singles = ctx.enter_context(tc.tile_pool(name="singles", bufs=1))
    stp = ctx.enter_context(tc.tile_pool(name="stp", bufs=28))
    nat16 = ctx.enter_context(tc.tile_pool(name="nat16", bufs=2))
    nat8 = ctx.enter_context(tc.tile_pool(name="nat8", bufs=2))
    psA = ctx.enter_context(tc.tile_pool(name="psA", bufs=1, space="PSUM"))
    psB = ctx.enter_context(tc.tile_pool(name="psB", bufs=2, space="PSUM"))
    psC = ctx.enter_context(tc.tile_pool(name="psC", bufs=2, space="PSUM"))

    ident = singles.tile([P, P], F32)
    make_identity(nc, ident)
    ident16 = singles.tile([P, P], F16)
    nc.vector.tensor_copy(out=ident16[:], in_=ident[:])
    ones8 = singles.tile([1, 8], F16)
    nc.vector.memset(ones8[:], 1.0)
    sh1_i = singles.tile([P, 1], I32)
    nc.vector.memset(sh1_i[:], 1)
    magic_i = singles.tile([P, 1], I32)
    nc.vector.memset(magic_i[:], 0x5F3759DF)

    def evict(dst, src):
        nc.vector.tensor_copy(out=dst, in_=src)

    def t_nat_to_T(src_nat, dst_T, nchunks, npart, uid, c0=0):
        """(npart, nchunks*128) sbuf -> (128, [c0+..], npart) sbuf via PE."""
        idn = ident16 if src_nat.dtype == F16 else ident
        for c in range(nchunks):
            ps = psB.tile([P, 8], src_nat.dtype, tag="tp8", name=f"tp_{uid}_{c}")
            nc.tensor.transpose(ps[:, :npart], src_nat[:, c * P:(c + 1) * P],
                                idn[:npart, :npart])
            evict(dst_T[:, c0 + c, :], ps[:, :npart])

    def bias_mm(ps, bias_row, n_total, stop=True):
        """Add a (1, n_total) f16 bias row into psum (8, n_total) via ones-row
        matmuls, 512 cols per matmul (moving-dim limit)."""
        nch = (n_total + 511) // 512
        for n in range(nch):
            w = min(512, n_total - n * 512)
            nc.tensor.matmul(ps[:, n * 512:n * 512 + w], ones8[:, :B],
                             bias_row[:, n * 512:n * 512 + w],
                             start=False, stop=(stop and n == nch - 1))

    def layernorm_nat(x_nat, npart, n, y_nat, uid, nchunks=1):
        """y = (x - mean) / sqrt(var + eps) over the free dim of (npart, n).
        rsqrt runs on VectorE (Quake bit-trick + 2 Newton steps) to avoid
        ACT Sqrt table loads. Output written in nchunks pieces so consumers
        (transposes) can start early."""
        nsub = max(1, n // 512)
        st = nat8.tile([npart, nsub, nc.vector.BN_STATS_DIM], F32, tag="lnst",
                       name=f"lnst_{uid}")
        xg = x_nat.rearrange("p (a b) -> p a b", a=nsub)
        for g in range(nsub):
            nc.vector.bn_stats(out=st[:, g, :], in_=xg[:, g, :])
        mv = nat8.tile([npart, nc.vector.BN_AGGR_DIM], F32, tag="lnmv",
                       name=f"lnmv_{uid}")
        nc.vector.bn_aggr(out=mv[:], in_=st[:])
        ve = nat8.tile([npart, 1], F32, tag="lnve", name=f"lnve_{uid}")
        nc.vector.tensor_scalar_add(out=ve[:], in0=mv[:, 1:2], scalar1=1e-5)
        yi = nat8.tile([npart, 1], I32, tag="lnyi", name=f"lnyi_{uid}")
        nc.vector.tensor_tensor(out=yi[:], in0=ve[:].bitcast(I32),
                                in1=sh1_i[:npart, :],
                                op=ALU.logical_shift_right)
        nc.vector.tensor_tensor(out=yi[:], in0=magic_i[:npart, :], in1=yi[:],
                                op=ALU.subtract)
        y = yi[:].bitcast(F32)
        tt = nat8.tile([npart, 1], F32, tag="lntt", name=f"lntt_{uid}")
        for _ in range(2):
            nc.vector.tensor_mul(out=tt[:], in0=y, in1=y)
            nc.vector.tensor_mul(out=tt[:], in0=tt[:], in1=ve[:])
            nc.vector.tensor_scalar(out=tt[:], in0=tt[:], scalar1=-0.5,
                                    scalar2=1.5, op0=ALU.mult, op1=ALU.add)
            nc.vector.tensor_mul(out=yi[:].bitcast(F32), in0=y, in1=tt[:])
        cw = n // nchunks
        for c in range(nchunks):
            nc.vector.tensor_scalar(out=y_nat[:, c * cw:(c + 1) * cw],
                                    in0=x_nat[:, c * cw:(c + 1) * cw],
                                    scalar1=mv[:, 0:1], scalar2=y,
                                    op0=ALU.subtract, op1=ALU.mult)

    # =======================================================================
    # STEP 0: constants, bias rows — prefetched early on queues that are
    # otherwise idle so later phases never wait on them.
    # =======================================================================
    u_sb = singles.tile([P, DC, H], F16)
    nc.sync.dma_start(out=u_sb[:], in_=t["u_r"][:])
    bv_sb = singles.tile([1, DH], F16)
    nc.gpsimd.dma_start(out=bv_sb[:], in_=t["bv16"][:])
    bo_sb = singles.tile([1, D], F16)
    nc.gpsimd.dma_start(out=bo_sb[:], in_=t["bo16"][:])
    b1_sb = singles.tile([1, F1S], F16)
    nc.gpsimd.dma_start(out=b1_sb[:], in_=t["b116"][:])
    b2_sb = singles.tile([1, D], F16)
    nc.gpsimd.dma_start(out=b2_sb[:], in_=t["b216"][:])
    rb_sb = singles.tile([1, HID], F16)
    nc.gpsimd.dma_start(out=rb_sb[:], in_=t["rb16"][:])
    bb1_sb = singles.tile([1, NBLK, 4 * HID], F16)
    nc.gpsimd.dma_start(out=bb1_sb[:], in_=t["blk_b1_16"][:].rearrange("n f -> (n f)")[None, :])
    bb2_sb = singles.tile([1, NBLK, HID], F16)
    nc.gpsimd.dma_start(out=bb2_sb[:], in_=t["blk_b2_16"][:].rearrange("n f -> (n f)")[None, :])
    rc_sb = singles.tile([TD, HID], F16)
    nc.gpsimd.dma_start(out=rc_sb[:], in_=t["rin_cond8"][:])
    rna_sb = singles.tile([AD, HID], F16)
    nc.gpsimd.dma_start(out=rna_sb[:], in_=t["rin_na8"][:])
    naT_sb = singles.tile([AD, B], F16)
    nc.sync.dma_start(out=naT_sb[:], in_=t["naT"][:])
    ow_sb = singles.tile([P, HC, AD], F16)
    nc.sync.dma_start(out=ow_sb[:],
                      in_=t["out_w"][:].rearrange("(c p) a -> p c a", p=P))
    ob_bc = singles.tile([B, AD], F32)
    nc.gpsimd.dma_start(out=ob_bc[:], in_=_bcast(t["out_bc"][:], B))

    # ---- cond path (fourier + tiny mlp) — independent of everything else.
    fw_sb = singles.tile([TD, 1], F32)
    nc.sync.dma_start(out=fw_sb[:], in_=t["four_w2"][:])
    ph_sb = singles.tile([TD, 1], F32)
    nc.sync.dma_start(out=ph_sb[:], in_=t["phase2"][:])
    tb32 = singles.tile([TD, B], F32)
    nc.gpsimd.dma_start(out=tb32[:], in_=_bcast(t["timeT"][:], TD))
    fu = singles.tile([TD, B], F32)
    nc.vector.tensor_scalar_mul(out=fu[:], in0=tb32[:], scalar1=fw_sb[:])
    # exact range reduction: sin/cos have period 1 in fu, so subtract the
    # integer part via an f32->i32->f32 round-trip (|fu| < ~64 here).
    fi = singles.tile([TD, B], I32)
    nc.vector.tensor_copy(out=fi[:], in_=fu[:])
    fif = singles.tile([TD, B], F32)
    nc.vector.tensor_copy(out=fif[:], in_=fi[:])
    nc.vector.tensor_sub(out=fu[:], in0=fu[:], in1=fif[:])
    ffT = singles.tile([TD, B], F16)
    nc.scalar.activation(out=ffT[:], in_=fu[:], func=AF.Sin,
                         scale=TWO_PI, bias=ph_sb[:])
    cw1_sb = singles.tile([TD, 2 * TD], F16)
    nc.scalar.dma_start(out=cw1_sb[:], in_=t["cond_w1"][:])
    cb1_sb = singles.tile([2 * TD, 1], F32)
    nc.sync.dma_start(out=cb1_sb[:], in_=t["cond_b1c"][:])
    cw2_sb = singles.tile([2 * TD, TD], F16)
    nc.scalar.dma_start(out=cw2_sb[:], in_=t["cond_w2"][:])
    cb2_sb = singles.tile([TD, 1], F32)
    nc.sync.dma_start(out=cb2_sb[:], in_=t["cond_b2c"][:])
    ps_c1 = psB.tile([P, 8], F32, tag="tp8", name="ps_c1")
    nc.tensor.matmul(ps_c1[:2 * TD, :B], cw1_sb[:], ffT[:], start=True, stop=True)
    c1 = singles.tile([2 * TD, B], F16)
    nc.scalar.activation(out=c1[:], in_=ps_c1[:2 * TD, :B], func=AF.Silu,
                         bias=cb1_sb[:])
    ps_c2 = psB.tile([P, 8], F32, tag="tp8", name="ps_c2")
    nc.tensor.matmul(ps_c2[:TD, :B], cw2_sb[:], c1[:], start=True, stop=True)
    condT = singles.tile([TD, B], F16)
    nc.scalar.activation(out=condT[:], in_=ps_c2[:TD, :B], func=AF.Identity,
                         bias=cb2_sb[:])

    # =======================================================================
    # STEP 1: scoresT (8, 2048) = U.T @ llmT  (fp16 inputs, fp32 accum)
    # =======================================================================
    ps_sc = psA.tile([H, S], F32, tag="big", name="ps_sc")
    for k in range(DC):
        lt = stp.tile([P, S], F16, tag="st", name=f"llmT_t{k}")
        nc.sync.dma_start(out=lt[:], in_=t["llmT"][k * P:(k + 1) * P, :])
        for n in range(S // 512):
            nc.tensor.matmul(ps_sc[:, n * 512:(n + 1) * 512],
                             u_sb[:, k, :], lt[:, n * 512:(n + 1) * 512],
                             start=(k == 0), stop=(k == DC - 1))

    # =======================================================================
    # STEP 2: softmax over S. Max-subtraction skipped deliberately: softmax
    # is shift-invariant and |scores| < ~1 here, so exp() is perfectly
    # conditioned; the result is mathematically identical.
    # =======================================================================
    p_nat = nat8.tile([H, S], F32, tag="nat8", name="p_nat")
    nc.scalar.activation(out=p_nat[:], in_=ps_sc[:], func=AF.Exp)
    den = singles.tile([H, 1], F32)
    nc.vector.reduce_sum(out=den[:], in_=p_nat[:], axis=mybir.AxisListType.X)
    nc.vector.reciprocal(out=den[:], in_=den[:])
    nc.vector.tensor_scalar_mul(out=p_nat[:], in0=p_nat[:], scalar1=den[:])
    pT = singles.tile([P, SC, H], F16)
    t_nat_to_T(p_nat, pT, SC, H, "p")

    # =======================================================================
    # STEP 3: pooled (8, 4096) = pT.T @ llm, by D-half; AllToAll per half
    # (head <-> batch) so A2A of half 0 overlaps half 1's matmuls.
    # =======================================================================
    for half in range(2):
        ps_p = psA.tile([H, HD2], F32, tag="big", name=f"ps_pool_{half}")
        for s in range(SC):
            lt = stp.tile([P, HD2], F16, tag="st", name=f"llm_t{half}_{s}")
            nc.sync.dma_start(
                out=lt[:],
                in_=t["llm"][s * P:(s + 1) * P, half * HD2:(half + 1) * HD2])
            for n in range(HD2 // 512):
                nc.tensor.matmul(ps_p[:, n * 512:(n + 1) * 512],
                                 pT[:, s, :], lt[:, n * 512:(n + 1) * 512],
                                 start=(s == 0), stop=(s == SC - 1))
        pooled_h = nat16.tile([H, HD2], F16, tag="nat16", name=f"pooled_{half}")
        evict(pooled_h[:], ps_p[:])
        nc.sync.dma_start(out=t[f"cc_pool_in{half}"][:], in_=pooled_h[:])
        nc.gpsimd.collective_compute(
            "AllToAll", ALU.bypass, replica_groups=GROUPS,
            ins=[t[f"cc_pool_in{half}"][:].opt()],
            outs=[t[f"cc_pool_out{half}"][:].opt()])

    # =======================================================================
    # STEP 4: ctx for this core's head, all batches: (8, 512) = poolh@wv + bv
    # accumulated per A2A half so half 0 overlaps half 1's collective.
    # =======================================================================
    poolhT = singles.tile([P, DC, B], F16)
    ps_cx = psC.tile([B, DH], F32, tag="vec", name="ps_cx")
    wv_r = t["wv_s"].rearrange("(c p) n -> p c n", p=P)
    for half in range(2):
        poolh_h = nat16.tile([B, HD2], F16, tag="nat16", name=f"poolh_{half}")
        nc.sync.dma_start(out=poolh_h[:], in_=t[f"cc_pool_out{half}"][:])
        t_nat_to_T(poolh_h, poolhT, SC, B, f"ph{half}", c0=half * SC)
        for g in range(4):
            gg = half * 4 + g
            wt = stp.tile([P, 4, DH], F16, tag="st", name=f"wv_g{gg}")
            nc.scalar.dma_start(out=wt[:], in_=wv_r[:, 4 * gg:4 * gg + 4, :])
            for j in range(4):
                k = 4 * gg + j
                nc.tensor.matmul(ps_cx[:], poolhT[:, k, :], wt[:, j, :],
                                 start=(k == 0), stop=False)
    bias_mm(ps_cx, bv_sb, DH)
    ctx_nat = nat8.tile([B, DH], F32, tag="nat8", name="ctx_nat")
    evict(ctx_nat[:], ps_cx[:])
    ctxT = singles.tile([P, DH // P, B], F16)
    t_nat_to_T(ctx_nat, ctxT, DH // P, B, "cx")

    # =======================================================================
    # STEP 5: attn partial (8, 4096) = ctx @ wo_s + bo/8 ; AllReduce (f16)
    # =======================================================================
    attn_part = nat16.tile([B, D], F16, tag="nat16", name="attn_part")
    for half in range(2):
        ps_a = psA.tile([B, HD2], F32, tag="big", name=f"ps_attn_{half}")
        for k in range(DH // P):
            wt = stp.tile([P, HD2], F16, tag="st", name=f"wo_t{half}_{k}")
            nc.scalar.dma_start(
                out=wt[:],
                in_=t["wo_s"][k * P:(k + 1) * P, half * HD2:(half + 1) * HD2])
            for n in range(HD2 // 512):
                nc.tensor.matmul(ps_a[:, n * 512:(n + 1) * 512],
                                 ctxT[:, k, :], wt[:, n * 512:(n + 1) * 512],
                                 start=(k == 0), stop=False)
        bias_mm(ps_a, bo_sb[:, half * HD2:(half + 1) * HD2], HD2)
        evict(attn_part[:, half * HD2:(half + 1) * HD2], ps_a[:])
    nc.sync.dma_start(out=t["cc_attn_in"][:], in_=attn_part[:])
    nc.gpsimd.collective_compute(
        "AllReduce", ALU.add, replica_groups=GROUPS,
        ins=[t["cc_attn_in"][:].opt()], outs=[t["cc_attn_out"][:].opt()])

    # =======================================================================
    # STEP 6: y = LN(attn_out) ; mlp partial h2 = gelu(y@w1'+b1')@w2 + b2/8
    # (LN affine folded into w1'/b1' on host)
    # =======================================================================
    attn_nat = singles.tile([B, D], F16)  # persists (residual)
    nc.sync.dma_start(out=attn_nat[:], in_=t["cc_attn_out"][:])
    # attn_out/8 staged into sum_pre now (cheap, off critical path); the mm2
    # partials are added in-place per half below.
    sum_pre = nat16.tile([B, D], F16, tag="nat16", name="sum_pre")
    nc.vector.tensor_scalar_mul(out=sum_pre[:], in0=attn_nat[:], scalar1=0.125)

    y_nat = nat16.tile([B, D], F16, tag="nat16", name="y_nat")
    layernorm_nat(attn_nat[:], B, D, y_nat[:], "ln0", nchunks=4)
    yT = singles.tile([P, DC, B], F16)
    t_nat_to_T(y_nat, yT, DC, B, "y")

    # mm1: h1 (8, 2048) = y @ w1' + b1' ; exact gelu straight off PSUM
    ps_h1 = psA.tile([B, F1S], F32, tag="big", name="ps_h1")
    for k in range(DC):
        wt = stp.tile([P, F1S], F16, tag="st", name=f"w1_t{k}")
        nc.scalar.dma_start(out=wt[:], in_=t["w1_s"][k * P:(k + 1) * P, :])
        for n in range(F1S // 512):
            nc.tensor.matmul(ps_h1[:, n * 512:(n + 1) * 512],
                             yT[:, k, :], wt[:, n * 512:(n + 1) * 512],
                             start=(k == 0), stop=False)
    bias_mm(ps_h1, b1_sb, F1S)
    g_nat = nat8.tile([B, F1S], F32, tag="nat8", name="g_nat")
    nc.scalar.activation(out=g_nat[:], in_=ps_h1[:], func=AF.Gelu)
    gT = singles.tile([P, F1S // P, B], F16)
    t_nat_to_T(g_nat, gT, F1S // P, B, "g")

    # mm2: h2 partial (8, 4096) = g @ w2_s + b2/8, accumulated into sum_pre
    for half in range(2):
        ps_h2 = psA.tile([B, HD2], F32, tag="big", name=f"ps_h2_{half}")
        for k in range(F1S // P):
            wt = stp.tile([P, HD2], F16, tag="st", name=f"w2_t{half}_{k}")
            nc.scalar.dma_start(
                out=wt[:],
                in_=t["w2_s"][k * P:(k + 1) * P, half * HD2:(half + 1) * HD2])
            for n in range(HD2 // 512):
                nc.tensor.matmul(ps_h2[:, n * 512:(n + 1) * 512],
                                 gT[:, k, :], wt[:, n * 512:(n + 1) * 512],
                                 start=(k == 0), stop=False)
        bias_mm(ps_h2, b2_sb[:, half * HD2:(half + 1) * HD2], HD2)
        nc.vector.tensor_add(
            out=sum_pre[:, half * HD2:(half + 1) * HD2],
            in0=sum_pre[:, half * HD2:(half + 1) * HD2], in1=ps_h2[:])

    spT = singles.tile([P, DC, B], F16)
    t_nat_to_T(sum_pre, spT, DC, B, "sp")

    # z (8, 256) = sum_pre@rp + cond@(rc/8) + na@(rna/8) + rb/8 ; AllReduce.
    # rp + tail block weights stream through the ring during mm2.
    rp_tiles = []
    for g in range(4):
        rpt = stp.tile([P, 8, HID], F16, tag="st", name=f"rp_g{g}")
        nc.scalar.dma_start(out=rpt[:], in_=t["rp_r"][:, 8 * g:8 * (g + 1), :])
        rp_tiles.append(rpt)
    bw1_tiles, bw2_tiles = [], []
    for i in range(NBLK):
        bt1 = stp.tile([P, HC, 4 * HID], F16, tag="st", name=f"bw1_{i}")
        nc.scalar.dma_start(out=bt1[:], in_=t["bw1_r"][:, i, :, :])
        bw1_tiles.append(bt1)
        bt2 = stp.tile([P, 4 * HID // P, HID], F16, tag="st", name=f"bw2_{i}")
        nc.scalar.dma_start(out=bt2[:], in_=t["bw2_r"][:, i, :, :])
        bw2_tiles.append(bt2)

    ps_z = psC.tile([B, HID], F32, tag="vec", name="ps_z")
    for k in range(DC):
        nc.tensor.matmul(ps_z[:], spT[:, k, :], rp_tiles[k // 8][:, k % 8, :],
                         start=(k == 0), stop=False)
    nc.tensor.matmul(ps_z[:], condT[:], rc_sb[:], start=False, stop=False)
    nc.tensor.matmul(ps_z[:], naT_sb[:], rna_sb[:], start=False, stop=False)
    bias_mm(ps_z, rb_sb, HID)
    z_nat = nat8.tile([B, HID], F32, tag="nat8", name="z_nat")
    evict(z_nat[:], ps_z[:])
    nc.sync.dma_start(out=t["cc_z_in"][:], in_=z_nat[:])
    nc.gpsimd.collective_compute(
        "AllReduce", ALU.add, replica_groups=GROUPS,
        ins=[t["cc_z_in"][:].opt()], outs=[t["cc_z_out"][:].opt()])

    # =======================================================================
    # STEP 7: diffusion tail (replicated on all cores; LN affines folded
    # into bw1/bb1 on host)
    # =======================================================================
    x_nat = singles.tile([B, HID], F32)
    nc.sync.dma_start(out=x_nat[:], in_=t["cc_z_out"][:])

    # ---- 3 residual blocks ----
    for i in range(NBLK):
        xn = singles.tile([B, HID], F32, name=f"xn_{i}")
        layernorm_nat(x_nat[:], B, HID, xn[:], f"lnb{i}")
        xnT = singles.tile([P, HC, B], F16, name=f"xnT_{i}")
        t_nat_to_T(xn, xnT, HC, B, f"xn{i}")

        ps_bh = psA.tile([B, 4 * HID], F32, tag="big", name=f"ps_bh_{i}")
        for k in range(HC):
            for n in range(4 * HID // 512):
                nc.tensor.matmul(ps_bh[:, n * 512:(n + 1) * 512],
                                 xnT[:, k, :],
                                 bw1_tiles[i][:, k, n * 512:(n + 1) * 512],
                                 start=(k == 0), stop=False)
        bias_mm(ps_bh, bb1_sb[:, i, :], 4 * HID)
        hb = nat8.tile([B, 4 * HID], F32, tag="nat8", name=f"hb_{i}")
        nc.scalar.activation(out=hb[:], in_=ps_bh[:], func=AF.Silu)
        hbT = singles.tile([P, 4 * HID // P, B], F16, name=f"hbT_{i}")
        t_nat_to_T(hb, hbT, 4 * HID // P, B, f"hb{i}")

        ps_bo = psC.tile([B, HID], F32, tag="vec", name=f"ps_bo_{i}")
        for k in range(4 * HID // P):
            nc.tensor.matmul(ps_bo[:], hbT[:, k, :], bw2_tiles[i][:, k, :],
                             start=(k == 0), stop=False)
        bias_mm(ps_bo, bb2_sb[:, i, :], HID)
        nc.vector.tensor_add(out=x_nat[:], in0=x_nat[:], in1=ps_bo[:])

    # ---- final: res (8, 7) = swish(x) @ out_w + out_b
    nc.scalar.activation(out=x_nat[:], in_=x_nat[:], func=AF.Silu)
    xsT = singles.tile([P, HC, B], F16)
    t_nat_to_T(x_nat, xsT, HC, B, "xs")
    ps_o = psB.tile([P, 8], F32, tag="tp8", name="ps_o")
    for k in range(HC):
        nc.tensor.matmul(ps_o[:B, :AD], xsT[:, k, :], ow_sb[:, k, :],
                         start=(k == 0), stop=(k == HC - 1))
    out_sb = singles.tile([B, AD], F32)
    nc.vector.tensor_add(out=out_sb[:], in0=ps_o[:B, :AD], in1=ob_bc[:])
    nc.sync.dma_start(out=t["res"][:], in_=out_sb[:])


_CACHED_NC = None


def _get_nc():
    global _CACHED_NC
    if _CACHED_NC is None:
        _CACHED_NC = build_program()
    return _CACHED_NC


def _prep_in_maps(inputs):
    f32 = np.float32
    f16 = np.float16
    llm_full = np.ascontiguousarray(np.asarray(inputs["llm_output"], dtype=f32))
    wq = np.asarray(inputs["wq"], f32); wk = np.asarray(inputs["wk"], f32)
    wv = np.asarray(inputs["wv"], f32); wo = np.asarray(inputs["wo"], f32)
    bq = np.asarray(inputs["bq"], f32); bv = np.asarray(inputs["bv"], f32)
    bo = np.asarray(inputs["bo"], f32)
    w1 = np.asarray(inputs["mlp_w1"], f32); b1 = np.asarray(inputs["mlp_b1"], f32)
    w2 = np.asarray(inputs["mlp_w2"], f32); b2 = np.asarray(inputs["mlp_b2"], f32)
    ln_g = np.asarray(inputs["ln_g"], f32); ln_b = np.asarray(inputs["ln_b"], f32)
    rin_w = np.asarray(inputs["rin_w"], f32)
    probe = np.asarray(inputs["probe"], f32).reshape(D)

    # U = wk[:, hs] @ q[hs] / sqrt(DH) is a pure function of parameters
    # (the probe attention query is input-independent) -> folded here.
    q = probe @ wq + bq                       # (D,)
    U = np.empty((D, H), f32)
    for h in range(H):
        hs = slice(h * DH, (h + 1) * DH)
        U[:, h] = wk[:, hs] @ q[hs]
    U *= 1.0 / np.sqrt(DH)

    # LN affine fold: LN(x)*g+b @ W == LN(x) @ (g*W) + b@W
    w1g = w1 * ln_g[:, None]
    b1_fold = b1 + ln_b @ w1                  # (4*D,)

    blk_g = np.asarray(inputs["blk_ln_g"], f32)
    blk_b = np.asarray(inputs["blk_ln_b"], f32)
    blk_w1 = np.asarray(inputs["blk_w1"], f32)   # (NBLK, HID, 4*HID)
    blk_w2 = np.asarray(inputs["blk_w2"], f32)   # (NBLK, 4*HID, HID)
    blk_b1 = np.asarray(inputs["blk_b1"], f32)
    bw1g = blk_w1 * blk_g[:, :, None]
    bb1_fold = blk_b1 + np.einsum("nh,nhf->nf", blk_b, blk_w1)

    shared = {
        "u_r": np.ascontiguousarray(
            U.reshape(DC, P, H).transpose(1, 0, 2)).astype(f16),
        "bo16": (bo / NC).astype(f16).reshape(1, D),
        "b216": (b2 / NC).astype(f16).reshape(1, D),
        "four_w2": np.concatenate(
            [np.asarray(inputs["four_w"], f32).reshape(TD // 2, 1)] * 2),
        "phase2": np.concatenate(
            [np.full((TD // 2, 1), np.pi / 2, f32),
             np.zeros((TD // 2, 1), f32)]),
        "timeT": np.ascontiguousarray(np.asarray(inputs["time"], f32).T),
        "naT": np.ascontiguousarray(
            np.asarray(inputs["noisy_actions"], f32).T).astype(f16),
        "cond_w1": np.asarray(inputs["cond_w1"], f32).astype(f16),
        "cond_b1c": np.asarray(inputs["cond_b1"], f32).reshape(-1, 1),
        "cond_w2": np.asarray(inputs["cond_w2"], f32).astype(f16),
        "cond_b2c": np.asarray(inputs["cond_b2"], f32).reshape(-1, 1),
        "rin_cond8": (np.ascontiguousarray(rin_w[0:TD]) / NC).astype(f16),
        "rp_r": np.ascontiguousarray(
            rin_w[TD:TD + D].reshape(DC, P, HID).transpose(1, 0, 2)
        ).astype(f16),
        "rin_na8": (np.ascontiguousarray(rin_w[TD + D:]) / NC).astype(f16),
        "rb16": (np.asarray(inputs["rin_b"], f32) / NC
                 ).astype(f16).reshape(1, HID),
        "bw1_r": np.ascontiguousarray(
            bw1g.reshape(NBLK, HC, P, 4 * HID).transpose(2, 0, 1, 3)
        ).astype(f16),
        "blk_b1_16": bb1_fold.astype(f16),
        "bw2_r": np.ascontiguousarray(
            blk_w2.reshape(NBLK, 4 * HID // P, P, HID).transpose(2, 0, 1, 3)
        ).astype(f16),
        "blk_b2_16": np.asarray(inputs["blk_b2"], f32).astype(f16),
        "out_w": np.asarray(inputs["out_w"], f32).astype(f16),
        "out_bc": np.asarray(inputs["out_b"], f32).reshape(1, AD),
    }

    in_maps = []
    for i in range(NC):
        hb = slice(i * DH, (i + 1) * DH)
        fb = slice(i * F1S, (i + 1) * F1S)
        m = dict(shared)
        m["llm"] = llm_full[i].astype(f16)
        m["llmT"] = np.ascontiguousarray(llm_full[i].T).astype(f16)
        m["wv_s"] = np.ascontiguousarray(wv[:, hb]).astype(f16)
        m["bv16"] = np.ascontiguousarray(bv[hb]).astype(f16).reshape(1, DH)
        m["wo_s"] = np.ascontiguousarray(wo[hb, :]).astype(f16)
        m["w1_s"] = np.ascontiguousarray(w1g[:, fb]).astype(f16)
        m["b116"] = np.ascontiguousarray(b1_fold[fb]).astype(f16).reshape(1, F1S)
        m["w2_s"] = np.ascontiguousarray(w2[fb, :]).astype(f16)
        in_maps.append(m)
    return in_maps


def kernel(**inputs):
    nc = _get_nc()
    in_maps = _prep_in_maps(inputs)
    r = run_bass_kernel_spmd(nc, in_maps, core_ids=list(range(NC)))
    return np.ascontiguousarray(r.results[0]["res"]).astype(np.float32)


def run_traced(**inputs):
    """Like kernel() but with NTFF tracing; returns (output, results)."""
    nc = _get_nc()
    in_maps = _prep_in_maps(inputs)
    r = run_bass_kernel_spmd(nc, in_maps, core_ids=list(range(NC)), trace=True)
    return np.ascontiguousarray(r.results[0]["res"]).astype(np.float32), r


# revision 12
# speedup vs baseline: 1.4300x; 1.1521x over previous
"""Trainium2 Bass kernel for nn_DiffusionActionHead (B=8, S=2048, D=4096).

Strategy (8 NeuronCores):
  - Data-parallel over batch for everything touching llm_output; tensor-
    parallel weight reads (core i: head-slice i of wv/wo, hidden-slice i of
    mlp_w1/mlp_w2), tiny diffusion tail replicated.
  - MAP-head attention with q_len=1 collapsed algebraically:
        scores[s,h] = llm[s,:] . U[:,h]
        pooled[h,:] = softmax(scores)[h,:] @ llm
    U = wk[:,h-block] @ q_h / sqrt(DH) is input-independent (probe/wq/bq/wk
    are all parameters), so U is folded on the host -> no wq/wk streams, no
    AllGather.  (bk shifts scores by a per-head constant -> cancels in
    softmax.)
  - 4 collectives: 2x AllToAll (pooled f16, split by D-half so the first
    A2A overlaps the second half's matmuls), AllReduce(attn partial f16),
    AllReduce(x0 partial (B,256) -- the rin_w projection is folded through
    the mlp AllReduce by linearity, shrinking payload 128KB -> 8KB).
  - Large matmuls run in fp16 (accumulation fp32); softmax/LN stats fp32.
  - LN affine (gamma/beta) is folded into w1/b1 (and blk_w1/blk_b1) on the
    host: LN(x)*g+b @ W == LN(x) @ (g*W) + (b@W + bias).
  - 1/sqrt(var+eps) computed on VectorE via Quake bit-trick + 2 Newton
    iterations -- avoids ACT Sqrt table swaps (1.3us each) in the tail.
  - Biases folded into the PSUM accumulations via a ones-row matmul;
    additive biases of AllReduce'd partials pre-divided by 8 on host.
  - ALL large streams (llmT, llm, wv, wo, w1, w2, rin_pool, blk weights)
    share ONE 28-slot ring pool: slots freed by the attention phase are
    immediately reused for weight prefetch, so DMA never idles across the
    collective barriers.  llm streams ride the sync HWDGE ring, weights the
    scalar HWDGE ring.
"""

import numpy as np
import sys

if "/opt/trn_rl_repo" not in sys.path:
    sys.path.insert(0, "/opt/trn_rl_repo")

import concourse.bass as bass
import concourse.tile as tile
from concourse import bacc, mybir
from concourse.masks import make_identity
from concourse.bass_utils import run_bass_kernel_spmd

F32 = mybir.dt.float32
F16 = mybir.dt.float16
I32 = mybir.dt.int32
F8 = mybir.dt.float8e4
AF = mybir.ActivationFunctionType
ALU = mybir.AluOpType

B, S, D = 8, 2048, 4096
H, AD, TD, HID, NBLK = 8, 7, 32, 256, 3
DH = D // H            # 512
NC = 8                 # cores
P = 128
SC = S // P            # 16 S-chunks
DC = D // P            # 32 D-chunks
HD2 = D // 2           # 2048 (half width -> 4-bank PSUM tiles)
F1S = 4 * D // NC      # 2048 per-core hidden cols of mlp_w1
HC = HID // P          # 2
TWO_PI = 2.0 * float(np.pi)


def _bcast(src_ap, nparts):
    """Partition-broadcast a (1, N) DRAM AP to (nparts, N)."""
    ap = src_ap
    assert ap.shape[0] == 1, ap.shape
    return bass.AP(tensor=ap.tensor, offset=ap.offset,
                   ap=[[0, nparts]] + [list(x) for x in ap.ap[1:]])


def build_program():
    nc = bacc.Bacc("TRN2", target_bir_lowering=False, debug=False,
                   num_devices=NC)
    t = {}

    def din(name, shape, dtype=F32):
        t[name] = nc.dram_tensor(name, shape, dtype, kind="ExternalInput")

    din("llm", [S, D], F16); din("llmT", [D, S], F8)
    din("u_r", [P, DC, H], F8)
    din("sc_inv", [H, 1])
    din("wv_s", [D, DH], F16); din("bv16", [1, DH], F16)
    din("wo_s", [DH, D], F16); din("bo16", [1, D], F16)        # bo/8
    din("w1_s", [D, F1S], F16); din("b116", [1, F1S], F16)     # g-folded
    din("w2_s", [F1S, D], F16); din("b216", [1, D], F16)       # b2/8
    din("four_w2", [TD, 1]); din("phase2", [TD, 1])
    din("timeT", [1, B]); din("naT", [AD, B], F16)
    din("cond_w1", [TD, 2 * TD], F16); din("cond_b1c", [2 * TD, 1])
    din("cond_w2", [2 * TD, TD], F16); din("cond_b2c", [TD, 1])
    din("rin_cond8", [TD, HID], F16)           # rin_w[cond rows] / 8
    din("rp_r", [P, DC, HID], F16)             # rin_w[pooled rows] p-major
    din("rin_na8", [AD, HID], F16)             # rin_w[na rows] / 8
    din("rb16", [1, HID], F16)                 # rin_b / 8
    din("bw1_r", [P, NBLK, HC, 4 * HID], F16)  # g-folded
    din("blk_b1_16", [NBLK, 4 * HID], F16)     # b-folded
    din("bw2_r", [P, NBLK, 4 * HID // P, HID], F16)
    din("blk_b2_16", [NBLK, HID], F16)
    din("out_w", [HID, AD], F16); din("out_bc", [1, AD])
    t["res"] = nc.dram_tensor("res", [B, AD], F32, kind="ExternalOutput")

    # collective bounce buffers (internal DRAM; outputs in Shared space)
    t["cc_pool_in"] = nc.dram_tensor("cc_pool_in", [H, D], F16)
    t["cc_pool_out"] = nc.dram_tensor("cc_pool_out", [B, D], F16)
    t["cc_attn_in"] = nc.dram_tensor("cc_attn_in", [B, D], F16)
    t["cc_attn_out"] = nc.dram_tensor("cc_attn_out", [B, D], F16,
                                      addr_space="Shared")
    t["cc_z_in"] = nc.dram_tensor("cc_z_in", [B, HID], F32)
    t["cc_z_out"] = nc.dram_tensor("cc_z_out", [B, HID], F32,
                                   addr_space="Shared")

    with tile.TileContext(nc) as tc:
        import contextlib
        with contextlib.ExitStack() as ctx:
            _build(nc, tc, t, ctx)
    nc.finalize()
    return nc


def _build(nc, tc, t, ctx):
    GROUPS = [list(range(NC))]

    singles = ctx.enter_context(tc.tile_pool(name="singles", bufs=1))
    stp = ctx.enter_context(tc.tile_pool(name="stp", bufs=28))
    nat16 = ctx.enter_context(tc.tile_pool(name="nat16", bufs=2))
    nat8 = ctx.enter_context(tc.tile_pool(name="nat8", bufs=2))
    psA = ctx.enter_context(tc.tile_pool(name="psA", bufs=1, space="PSUM"))
    psB = ctx.enter_context(tc.tile_pool(name="psB", bufs=2, space="PSUM"))
    psC = ctx.enter_context(tc.tile_pool(name="psC", bufs=2, space="PSUM"))

    ident = singles.tile([P, P], F32)
    make_identity(nc, ident)
    ident16 = singles.tile([P, P], F16)
    nc.vector.tensor_copy(out=ident16[:], in_=ident[:])
    ones8 = singles.tile([1, 8], F16)
    nc.vector.memset(ones8[:], 1.0)
    sh1_i = singles.tile([P, 1], I32)
    nc.vector.memset(sh1_i[:], 1)
    magic_i = singles.tile([P, 1], I32)
    nc.vector.memset(magic_i[:], 0x5F3759DF)

    def evict(dst, src):
        nc.vector.tensor_copy(out=dst, in_=src)

    def t_nat_to_T(src_nat, dst_T, nchunks, npart, uid, c0=0):
        """(npart, nchunks*128) sbuf -> (128, [c0+..], npart) sbuf via PE."""
        idn = ident16 if src_nat.dtype == F16 else ident
        for c in range(nchunks):
            ps = psB.tile([P, 8], src_nat.dtype, tag="tp8", name=f"tp_{uid}_{c}")
            nc.tensor.transpose(ps[:, :npart], src_nat[:, c * P:(c + 1) * P],
                                idn[:npart, :npart])
            evict(dst_T[:, c0 + c, :], ps[:, :npart])

    def bias_mm(ps, bias_row, n_total, stop=True):
        """Add a (1, n_total) f16 bias row into psum (8, n_total) via ones-row
        matmuls, 512 cols per matmul (moving-dim limit)."""
        nch = (n_total + 511) // 512
        for n in range(nch):
            w = min(512, n_total - n * 512)
            nc.tensor.matmul(ps[:, n * 512:n * 512 + w], ones8[:, :B],
                             bias_row[:, n * 512:n * 512 + w],
                             start=False, stop=(stop and n == nch - 1))

    def layernorm_nat(x_nat, npart, n, y_nat, uid, nchunks=1):
        """y = (x - mean) / sqrt(var + eps) over the free dim of (npart, n).
        rsqrt runs on VectorE (Quake bit-trick + 2 Newton steps) to avoid
        ACT Sqrt table loads. Output written in nchunks pieces so consumers
        (transposes) can start early."""
        nsub = max(1, n // 512)
        st = nat8.tile([npart, nsub, nc.vector.BN_STATS_DIM], F32, tag="lnst",
                       name=f"lnst_{uid}")
        xg = x_nat.rearrange("p (a b) -> p a b", a=nsub)
        for g in range(nsub):
            nc.vector.bn_stats(out=st[:, g, :], in_=xg[:, g, :])
        mv = nat8.tile([npart, nc.vector.BN_AGGR_DIM], F32, tag="lnmv",
                       name=f"lnmv_{uid}")
        nc.vector.bn_aggr(out=mv[:], in_=st[:])
        ve = nat8.tile([npart, 1], F32, tag="lnve", name=f"lnve_{uid}")
        nc.vector.tensor_scalar_add(out=ve[:], in0=mv[:, 1:2], scalar1=1e-5)
        yi = nat8.tile([npart, 1], I32, tag="lnyi", name=f"lnyi_{uid}")
        nc.vector.tensor_tensor(out=yi[:], in0=ve[:].bitcast(I32),
                                in1=sh1_i[:npart, :],
                                op=ALU.logical_shift_right)
        nc.vector.tensor_tensor(out=yi[:], in0=magic_i[:npart, :], in1=yi[:],
                                op=ALU.subtract)
        y = yi[:].bitcast(F32)
        tt = nat8.tile([npart, 1], F32, tag="lntt", name=f"lntt_{uid}")
        for _ in range(2):
            nc.vector.tensor_mul(out=tt[:], in0=y, in1=y)
            nc.vector.tensor_mul(out=tt[:], in0=tt[:], in1=ve[:])
            nc.vector.tensor_scalar(out=tt[:], in0=tt[:], scalar1=-0.5,
                                    scalar2=1.5, op0=ALU.mult, op1=ALU.add)
            nc.vector.tensor_mul(out=yi[:].bitcast(F32), in0=y, in1=tt[:])
        cw = n // nchunks
        for c in range(nchunks):
            nc.vector.tensor_scalar(out=y_nat[:, c * cw:(c + 1) * cw],
                                    in0=x_nat[:, c * cw:(c + 1) * cw],
                                    scalar1=mv[:, 0:1], scalar2=y,
                                    op0=ALU.subtract, op1=ALU.mult)

    # =======================================================================
    # STEP 0: constants, bias rows — prefetched early on queues that are
    # otherwise idle so later phases never wait on them.
    # =======================================================================
    u_sb = singles.tile([P, DC, H], F8)
    nc.sync.dma_start(out=u_sb[:], in_=t["u_r"][:])
    sci_sb = singles.tile([H, 1], F32)
    nc.sync.dma_start(out=sci_sb[:], in_=t["sc_inv"][:])
    bv_sb = singles.tile([1, DH], F16)
    nc.gpsimd.dma_start(out=bv_sb[:], in_=t["bv16"][:])
    bo_sb = singles.tile([1, D], F16)
    nc.gpsimd.dma_start(out=bo_sb[:], in_=t["bo16"][:])
    b1_sb = singles.tile([1, F1S], F16)
    nc.gpsimd.dma_start(out=b1_sb[:], in_=t["b116"][:])
    b2_sb = singles.tile([1, D], F16)
    nc.gpsimd.dma_start(out=b2_sb[:], in_=t["b216"][:])
    rb_sb = singles.tile([1, HID], F16)
    nc.gpsimd.dma_start(out=rb_sb[:], in_=t["rb16"][:])
    bb1_sb = singles.tile([1, NBLK, 4 * HID], F16)
    nc.gpsimd.dma_start(out=bb1_sb[:], in_=t["blk_b1_16"][:].rearrange("n f -> (n f)")[None, :])
    bb2_sb = singles.tile([1, NBLK, HID], F16)
    nc.gpsimd.dma_start(out=bb2_sb[:], in_=t["blk_b2_16"][:].rearrange("n f -> (n f)")[None, :])
    rc_sb = singles.tile([TD, HID], F16)
    nc.gpsimd.dma_start(out=rc_sb[:], in_=t["rin_cond8"][:])
    rna_sb = singles.tile([AD, HID], F16)
    nc.gpsimd.dma_start(out=rna_sb[:], in_=t["rin_na8"][:])
    naT_sb = singles.tile([AD, B], F16)
    nc.sync.dma_start(out=naT_sb[:], in_=t["naT"][:])
    ow_sb = singles.tile([P, HC, AD], F16)
    nc.sync.dma_start(out=ow_sb[:],
                      in_=t["out_w"][:].rearrange("(c p) a -> p c a", p=P))
    ob_bc = singles.tile([B, AD], F32)
    nc.gpsimd.dma_start(out=ob_bc[:], in_=_bcast(t["out_bc"][:], B))

    # ---- cond path (fourier + tiny mlp) — independent of everything else.
    fw_sb = singles.tile([TD, 1], F32)
    nc.sync.dma_start(out=fw_sb[:], in_=t["four_w2"][:])
    ph_sb = singles.tile([TD, 1], F32)
    nc.sync.dma_start(out=ph_sb[:], in_=t["phase2"][:])
    tb32 = singles.tile([TD, B], F32)
    nc.gpsimd.dma_start(out=tb32[:], in_=_bcast(t["timeT"][:], TD))
    fu = singles.tile([TD, B], F32)
    nc.vector.tensor_scalar_mul(out=fu[:], in0=tb32[:], scalar1=fw_sb[:])
    # exact range reduction: sin/cos have period 1 in fu, so subtract the
    # integer part via an f32->i32->f32 round-trip (|fu| < ~64 here).
    fi = singles.tile([TD, B], I32)
    nc.vector.tensor_copy(out=fi[:], in_=fu[:])
    fif = singles.tile([TD, B], F32)
    nc.vector.tensor_copy(out=fif[:], in_=fi[:])
    nc.vector.tensor_sub(out=fu[:], in0=fu[:], in1=fif[:])
    ffT = singles.tile([TD, B], F16)
    nc.scalar.activation(out=ffT[:], in_=fu[:], func=AF.Sin,
                         scale=TWO_PI, bias=ph_sb[:])
    cw1_sb = singles.tile([TD, 2 * TD], F16)
    nc.scalar.dma_start(out=cw1_sb[:], in_=t["cond_w1"][:])
    cb1_sb = singles.tile([2 * TD, 1], F32)
    nc.sync.dma_start(out=cb1_sb[:], in_=t["cond_b1c"][:])
    cw2_sb = singles.tile([2 * TD, TD], F16)
    nc.scalar.dma_start(out=cw2_sb[:], in_=t["cond_w2"][:])
    cb2_sb = singles.tile([TD, 1], F32)
    nc.sync.dma_start(out=cb2_sb[:], in_=t["cond_b2c"][:])
    ps_c1 = psB.tile([P, 8], F32, tag="tp8", name="ps_c1")
    nc.tensor.matmul(ps_c1[:2 * TD, :B], cw1_sb[:], ffT[:], start=True, stop=True)
    c1 = singles.tile([2 * TD, B], F16)
    nc.scalar.activation(out=c1[:], in_=ps_c1[:2 * TD, :B], func=AF.Silu,
                         bias=cb1_sb[:])
    ps_c2 = psB.tile([P, 8], F32, tag="tp8", name="ps_c2")
    nc.tensor.matmul(ps_c2[:TD, :B], cw2_sb[:], c1[:], start=True, stop=True)
    condT = singles.tile([TD, B], F16)
    nc.scalar.activation(out=condT[:], in_=ps_c2[:TD, :B], func=AF.Identity,
                         bias=cb2_sb[:])

    # =======================================================================
    # STEP 1: scoresT (8, 2048) = U.T @ llmT  (fp16 inputs, fp32 accum)
    # =======================================================================
    ps_sc = psA.tile([H, S], F32, tag="big", name="ps_sc")
    for k in range(DC):
        lt = stp.tile([P, S], F8, tag="st", name=f"llmT_t{k}")
        nc.sync.dma_start(out=lt[:], in_=t["llmT"][k * P:(k + 1) * P, :])
        for n in range(S // 512):
            nc.tensor.matmul(ps_sc[:, n * 512:(n + 1) * 512],
                             u_sb[:, k, :], lt[:, n * 512:(n + 1) * 512],
                             start=(k == 0), stop=(k == DC - 1))

    # =======================================================================
    # STEP 2: softmax over S. Max-subtraction skipped deliberately: softmax
    # is shift-invariant and |scores| < ~1 here, so exp() is perfectly
    # conditioned; the result is mathematically identical.
    # =======================================================================
    p_nat = nat8.tile([H, S], F32, tag="nat8", name="p_nat")
    nc.scalar.activation(out=p_nat[:], in_=ps_sc[:], func=AF.Exp,
                         scale=sci_sb[:])
    den = singles.tile([H, 1], F32)
    nc.vector.reduce_sum(out=den[:], in_=p_nat[:], axis=mybir.AxisListType.X)
    nc.vector.reciprocal(out=den[:], in_=den[:])
    nc.vector.tensor_scalar_mul(out=p_nat[:], in0=p_nat[:], scalar1=den[:])
    pT = singles.tile([P, SC, H], F16)
    t_nat_to_T(p_nat, pT, SC, H, "p")

    # =======================================================================
    # STEP 3: pooled (8, 4096) = pT.T @ llm, by D-half; AllToAll per half
    # (head <-> batch) so A2A of half 0 overlaps half 1's matmuls.
    # =======================================================================
    pooled_nat = nat16.tile([H, D], F16, tag="nat16", name="pooled_nat")
    for half in range(2):
        ps_p = psA.tile([H, HD2], F32, tag="big", name=f"ps_pool_{half}")
        for s in range(SC):
            lt = stp.tile([P, HD2], F16, tag="st", name=f"llm_t{half}_{s}")
            nc.sync.dma_start(
                out=lt[:],
                in_=t["llm"][s * P:(s + 1) * P, half * HD2:(half + 1) * HD2])
            for n in range(HD2 // 512):
                nc.tensor.matmul(ps_p[:, n * 512:(n + 1) * 512],
                                 pT[:, s, :], lt[:, n * 512:(n + 1) * 512],
                                 start=(s == 0), stop=(s == SC - 1))
        evict(pooled_nat[:, half * HD2:(half + 1) * HD2], ps_p[:])
    nc.sync.dma_start(out=t["cc_pool_in"][:], in_=pooled_nat[:])
    nc.gpsimd.collective_compute(
        "AllToAll", ALU.bypass, replica_groups=GROUPS,
        ins=[t["cc_pool_in"][:].opt()], outs=[t["cc_pool_out"][:].opt()])

    # =======================================================================
    # STEP 4: ctx for this core's head, all batches: (8, 512) = poolh@wv + bv
    # accumulated per A2A half so half 0 overlaps half 1's collective.
    # =======================================================================
    poolh_nat = nat16.tile([B, D], F16, tag="nat16", name="poolh_nat")
    nc.sync.dma_start(out=poolh_nat[:], in_=t["cc_pool_out"][:])
    poolhT = singles.tile([P, DC, B], F16)
    t_nat_to_T(poolh_nat, poolhT, DC, B, "ph")
    ps_cx = psC.tile([B, DH], F32, tag="vec", name="ps_cx")
    wv_r = t["wv_s"].rearrange("(c p) n -> p c n", p=P)
    for g in range(8):
        wt = stp.tile([P, 4, DH], F16, tag="st", name=f"wv_g{g}")
        nc.scalar.dma_start(out=wt[:], in_=wv_r[:, 4 * g:4 * g + 4, :])
        for j in range(4):
            k = 4 * g + j
            nc.tensor.matmul(ps_cx[:], poolhT[:, k, :], wt[:, j, :],
                             start=(k == 0), stop=False)
    bias_mm(ps_cx, bv_sb, DH)
    ctx_nat = nat8.tile([B, DH], F32, tag="nat8", name="ctx_nat")
    evict(ctx_nat[:], ps_cx[:])
    ctxT = singles.tile([P, DH // P, B], F16)
    t_nat_to_T(ctx_nat, ctxT, DH // P, B, "cx")

    # =======================================================================
    # STEP 5: attn partial (8, 4096) = ctx @ wo_s + bo/8 ; AllReduce (f16)
    # =======================================================================
    attn_part = nat16.tile([B, D], F16, tag="nat16", name="attn_part")
    for half in range(2):
        ps_a = psA.tile([B, HD2], F32, tag="big", name=f"ps_attn_{half}")
        for k in range(DH // P):
            wt = stp.tile([P, HD2], F16, tag="st", name=f"wo_t{half}_{k}")
            nc.scalar.dma_start(
                out=wt[:],
                in_=t["wo_s"][k * P:(k + 1) * P, half * HD2:(half + 1) * HD2])
            for n in range(HD2 // 512):
                nc.tensor.matmul(ps_a[:, n * 512:(n + 1) * 512],
                                 ctxT[:, k, :], wt[:, n * 512:(n + 1) * 512],
                                 start=(k == 0), stop=False)
        bias_mm(ps_a, bo_sb[:, half * HD2:(half + 1) * HD2], HD2)
        evict(attn_part[:, half * HD2:(half + 1) * HD2], ps_a[:])
    nc.sync.dma_start(out=t["cc_attn_in"][:], in_=attn_part[:])
    nc.gpsimd.collective_compute(
        "AllReduce", ALU.add, replica_groups=GROUPS,
        ins=[t["cc_attn_in"][:].opt()], outs=[t["cc_attn_out"][:].opt()])

    # =======================================================================
    # STEP 6: y = LN(attn_out) ; mlp partial h2 = gelu(y@w1'+b1')@w2 + b2/8
    # (LN affine folded into w1'/b1' on host)
    # =======================================================================
    attn_nat = singles.tile([B, D], F16)  # persists (residual)
    nc.sync.dma_start(out=attn_nat[:], in_=t["cc_attn_out"][:])
    # attn_out/8 staged into sum_pre now (cheap, off critical path); the mm2
    # partials are added in-place per half below.
    sum_pre = nat16.tile([B, D], F16, tag="nat16", name="sum_pre")
    nc.vector.tensor_scalar_mul(out=sum_pre[:], in0=attn_nat[:], scalar1=0.125)

    y_nat = nat16.tile([B, D], F16, tag="nat16", name="y_nat")
    layernorm_nat(attn_nat[:], B, D, y_nat[:], "ln0", nchunks=4)
    yT = singles.tile([P, DC, B], F16)
    t_nat_to_T(y_nat, yT, DC, B, "y")

    # mm1: h1 (8, 2048) = y @ w1' + b1' ; exact gelu straight off PSUM
    ps_h1 = psA.tile([B, F1S], F32, tag="big", name="ps_h1")
    for k in range(DC):
        wt = stp.tile([P, F1S], F16, tag="st", name=f"w1_t{k}")
        nc.scalar.dma_start(out=wt[:], in_=t["w1_s"][k * P:(k + 1) * P, :])
        for n in range(F1S // 512):
            nc.tensor.matmul(ps_h1[:, n * 512:(n + 1) * 512],
                             yT[:, k, :], wt[:, n * 512:(n + 1) * 512],
                             start=(k == 0), stop=False)
    bias_mm(ps_h1, b1_sb, F1S)
    g_nat = nat8.tile([B, F1S], F32, tag="nat8", name="g_nat")
    nc.scalar.activation(out=g_nat[:], in_=ps_h1[:], func=AF.Gelu)
    gT = singles.tile([P, F1S // P, B], F16)
    t_nat_to_T(g_nat, gT, F1S // P, B, "g")

    # mm2: h2 partial (8, 4096) = g @ w2_s + b2/8, accumulated into sum_pre
    for half in range(2):
        ps_h2 = psA.tile([B, HD2], F32, tag="big", name=f"ps_h2_{half}")
        for k in range(F1S // P):
            wt = stp.tile([P, HD2], F16, tag="st", name=f"w2_t{half}_{k}")
            nc.scalar.dma_start(
                out=wt[:],
                in_=t["w2_s"][k * P:(k + 1) * P, half * HD2:(half + 1) * HD2])
            for n in range(HD2 // 512):
                nc.tensor.matmul(ps_h2[:, n * 512:(n + 1) * 512],
                                 gT[:, k, :], wt[:, n * 512:(n + 1) * 512],
                                 start=(k == 0), stop=False)
        bias_mm(ps_h2, b2_sb[:, half * HD2:(half + 1) * HD2], HD2)
        nc.vector.tensor_add(
            out=sum_pre[:, half * HD2:(half + 1) * HD2],
            in0=sum_pre[:, half * HD2:(half + 1) * HD2], in1=ps_h2[:])

    spT = singles.tile([P, DC, B], F16)
    t_nat_to_T(sum_pre, spT, DC, B, "sp")

    # z (8, 256) = sum_pre@rp + cond@(rc/8) + na@(rna/8) + rb/8 ; AllReduce.
    # rp + tail block weights stream through the ring during mm2.
    rp_tiles = []
    for g in range(4):
        rpt = stp.tile([P, 8, HID], F16, tag="st", name=f"rp_g{g}")
        nc.scalar.dma_start(out=rpt[:], in_=t["rp_r"][:, 8 * g:8 * (g + 1), :])
        rp_tiles.append(rpt)
    bw1_tiles, bw2_tiles = [], []
    for i in range(NBLK):
        bt1 = stp.tile([P, HC, 4 * HID], F16, tag="st", name=f"bw1_{i}")
        nc.scalar.dma_start(out=bt1[:], in_=t["bw1_r"][:, i, :, :])
        bw1_tiles.append(bt1)
        bt2 = stp.tile([P, 4 * HID // P, HID], F16, tag="st", name=f"bw2_{i}")
        nc.scalar.dma_start(out=bt2[:], in_=t["bw2_r"][:, i, :, :])
        bw2_tiles.append(bt2)

    ps_z = psC.tile([B, HID], F32, tag="vec", name="ps_z")
    for k in range(DC):
        nc.tensor.matmul(ps_z[:], spT[:, k, :], rp_tiles[k // 8][:, k % 8, :],
                         start=(k == 0), stop=False)
    nc.tensor.matmul(ps_z[:], condT[:], rc_sb[:], start=False, stop=False)
    nc.tensor.matmul(ps_z[:], naT_sb[:], rna_sb[:], start=False, stop=False)
    bias_mm(ps_z, rb_sb, HID)
    z_nat = nat8.tile([B, HID], F32, tag="nat8", name="z_nat")
    evict(z_nat[:], ps_z[:])
    nc.sync.dma_start(out=t["cc_z_in"][:], in_=z_nat[:])
    nc.gpsimd.collective_compute(
        "AllReduce", ALU.add, replica_groups=GROUPS,
        ins=[t["cc_z_in"][:].opt()], outs=[t["cc_z_out"][:].opt()])

    # =======================================================================
    # STEP 7: diffusion tail (replicated on all cores; LN affines folded
    # into bw1/bb1 on host)
    # =======================================================================
    x_nat = singles.tile([B, HID], F32)
    nc.sync.dma_start(out=x_nat[:], in_=t["cc_z_out"][:])

    # ---- 3 residual blocks ----
    for i in range(NBLK):
        xn = singles.tile([B, HID], F32, name=f"xn_{i}")
        layernorm_nat(x_nat[:], B, HID, xn[:], f"lnb{i}")
        xnT = singles.tile([P, HC, B], F16, name=f"xnT_{i}")
        t_nat_to_T(xn, xnT, HC, B, f"xn{i}")

        ps_bh = psA.tile([B, 4 * HID], F32, tag="big", name=f"ps_bh_{i}")
        for k in range(HC):
            for n in range(4 * HID // 512):
                nc.tensor.matmul(ps_bh[:, n * 512:(n + 1) * 512],
                                 xnT[:, k, :],
                                 bw1_tiles[i][:, k, n * 512:(n + 1) * 512],
                                 start=(k == 0), stop=False)
        bias_mm(ps_bh, bb1_sb[:, i, :], 4 * HID)
        hb = nat8.tile([B, 4 * HID], F32, tag="nat8", name=f"hb_{i}")
        nc.scalar.activation(out=hb[:], in_=ps_bh[:], func=AF.Silu)
        hbT = singles.tile([P, 4 * HID // P, B], F16, name=f"hbT_{i}")
        t_nat_to_T(hb, hbT, 4 * HID // P, B, f"hb{i}")

        ps_bo = psC.tile([B, HID], F32, tag="vec", name=f"ps_bo_{i}")
        for k in range(4 * HID // P):
            nc.tensor.matmul(ps_bo[:], hbT[:, k, :], bw2_tiles[i][:, k, :],
                             start=(k == 0), stop=False)
        bias_mm(ps_bo, bb2_sb[:, i, :], HID)
        nc.vector.tensor_add(out=x_nat[:], in0=x_nat[:], in1=ps_bo[:])

    # ---- final: res (8, 7) = swish(x) @ out_w + out_b
    nc.scalar.activation(out=x_nat[:], in_=x_nat[:], func=AF.Silu)
    xsT = singles.tile([P, HC, B], F16)
    t_nat_to_T(x_nat, xsT, HC, B, "xs")
    ps_o = psB.tile([P, 8], F32, tag="tp8", name="ps_o")
    for k in range(HC):
        nc.tensor.matmul(ps_o[:B, :AD], xsT[:, k, :], ow_sb[:, k, :],
                         start=(k == 0), stop=(k == HC - 1))
    out_sb = singles.tile([B, AD], F32)
    nc.vector.tensor_add(out=out_sb[:], in0=ps_o[:B, :AD], in1=ob_bc[:])
    nc.sync.dma_start(out=t["res"][:], in_=out_sb[:])


_CACHED_NC = None


def _get_nc():
    global _CACHED_NC
    if _CACHED_NC is None:
        _CACHED_NC = build_program()
    return _CACHED_NC


def _prep_in_maps(inputs):
    f32 = np.float32
    f16 = np.float16
    llm_full = np.ascontiguousarray(np.asarray(inputs["llm_output"], dtype=f32))
    wq = np.asarray(inputs["wq"], f32); wk = np.asarray(inputs["wk"], f32)
    wv = np.asarray(inputs["wv"], f32); wo = np.asarray(inputs["wo"], f32)
    bq = np.asarray(inputs["bq"], f32); bv = np.asarray(inputs["bv"], f32)
    bo = np.asarray(inputs["bo"], f32)
    w1 = np.asarray(inputs["mlp_w1"], f32); b1 = np.asarray(inputs["mlp_b1"], f32)
    w2 = np.asarray(inputs["mlp_w2"], f32); b2 = np.asarray(inputs["mlp_b2"], f32)
    ln_g = np.asarray(inputs["ln_g"], f32); ln_b = np.asarray(inputs["ln_b"], f32)
    rin_w = np.asarray(inputs["rin_w"], f32)
    probe = np.asarray(inputs["probe"], f32).reshape(D)

    # U = wk[:, hs] @ q[hs] / sqrt(DH) is a pure function of parameters
    # (the probe attention query is input-independent) -> folded here.
    q = probe @ wq + bq                       # (D,)
    U = np.empty((D, H), f32)
    for h in range(H):
        hs = slice(h * DH, (h + 1) * DH)
        U[:, h] = wk[:, hs] @ q[hs]
    U *= 1.0 / np.sqrt(DH)
    f8 = mybir.dt.np(F8)
    u_scale = float(2.0 ** np.floor(np.log2(64.0 / max(np.abs(U).max(), 1e-30))))

    # LN affine fold: LN(x)*g+b @ W == LN(x) @ (g*W) + b@W
    w1g = w1 * ln_g[:, None]
    b1_fold = b1 + ln_b @ w1                  # (4*D,)

    blk_g = np.asarray(inputs["blk_ln_g"], f32)
    blk_b = np.asarray(inputs["blk_ln_b"], f32)
    blk_w1 = np.asarray(inputs["blk_w1"], f32)   # (NBLK, HID, 4*HID)
    blk_w2 = np.asarray(inputs["blk_w2"], f32)   # (NBLK, 4*HID, HID)
    blk_b1 = np.asarray(inputs["blk_b1"], f32)
    bw1g = blk_w1 * blk_g[:, :, None]
    bb1_fold = blk_b1 + np.einsum("nh,nhf->nf", blk_b, blk_w1)

    shared = {
        "u_r": np.ascontiguousarray(
            (U * u_scale).reshape(DC, P, H).transpose(1, 0, 2)).astype(f8),
        "sc_inv": np.full((H, 1), 1.0 / u_scale, f32),
        "bo16": (bo / NC).astype(f16).reshape(1, D),
        "b216": (b2 / NC).astype(f16).reshape(1, D),
        "four_w2": np.concatenate(
            [np.asarray(inputs["four_w"], f32).reshape(TD // 2, 1)] * 2),
        "phase2": np.concatenate(
            [np.full((TD // 2, 1), np.pi / 2, f32),
             np.zeros((TD // 2, 1), f32)]),
        "timeT": np.ascontiguousarray(np.asarray(inputs["time"], f32).T),
        "naT": np.ascontiguousarray(
            np.asarray(inputs["noisy_actions"], f32).T).astype(f16),
        "cond_w1": np.asarray(inputs["cond_w1"], f32).astype(f16),
        "cond_b1c": np.asarray(inputs["cond_b1"], f32).reshape(-1, 1),
        "cond_w2": np.asarray(inputs["cond_w2"], f32).astype(f16),
        "cond_b2c": np.asarray(inputs["cond_b2"], f32).reshape(-1, 1),
        "rin_cond8": (np.ascontiguousarray(rin_w[0:TD]) / NC).astype(f16),
        "rp_r": np.ascontiguousarray(
            rin_w[TD:TD + D].reshape(DC, P, HID).transpose(1, 0, 2)
        ).astype(f16),
        "rin_na8": (np.ascontiguousarray(rin_w[TD + D:]) / NC).astype(f16),
        "rb16": (np.asarray(inputs["rin_b"], f32) / NC
                 ).astype(f16).reshape(1, HID),
        "bw1_r": np.ascontiguousarray(
            bw1g.reshape(NBLK, HC, P, 4 * HID).transpose(2, 0, 1, 3)
        ).astype(f16),
        "blk_b1_16": bb1_fold.astype(f16),
        "bw2_r": np.ascontiguousarray(
            blk_w2.reshape(NBLK, 4 * HID // P, P, HID).transpose(2, 0, 1, 3)
        ).astype(f16),
        "blk_b2_16": np.asarray(inputs["blk_b2"], f32).astype(f16),
        "out_w": np.asarray(inputs["out_w"], f32).astype(f16),
        "out_bc": np.asarray(inputs["out_b"], f32).reshape(1, AD),
    }

    in_maps = []
    for i in range(NC):
        hb = slice(i * DH, (i + 1) * DH)
        fb = slice(i * F1S, (i + 1) * F1S)
        m = dict(shared)
        m["llm"] = llm_full[i].astype(f16)
        m["llmT"] = np.ascontiguousarray(llm_full[i].T).astype(f8)
        m["wv_s"] = np.ascontiguousarray(wv[:, hb]).astype(f16)
        m["bv16"] = np.ascontiguousarray(bv[hb]).astype(f16).reshape(1, DH)
        m["wo_s"] = np.ascontiguousarray(wo[hb, :]).astype(f16)
        m["w1_s"] = np.ascontiguousarray(w1g[:, fb]).astype(f16)
        m["b116"] = np.ascontiguousarray(b1_fold[fb]).astype(f16).reshape(1, F1S)
        m["w2_s"] = np.ascontiguousarray(w2[fb, :]).astype(f16)
        in_maps.append(m)
    return in_maps


def kernel(**inputs):
    nc = _get_nc()
    in_maps = _prep_in_maps(inputs)
    r = run_bass_kernel_spmd(nc, in_maps, core_ids=list(range(NC)))
    return np.ascontiguousarray(r.results[0]["res"]).astype(np.float32)


def run_traced(**inputs):
    """Like kernel() but with NTFF tracing; returns (output, results)."""
    nc = _get_nc()
    in_maps = _prep_in_maps(inputs)
    r = run_bass_kernel_spmd(nc, in_maps, core_ids=list(range(NC)), trace=True)
    return np.ascontiguousarray(r.results[0]["res"]).astype(np.float32), r
